# revision 1
# baseline (speedup 1.0000x reference)
"""GNN message-passing (graph convolution) kernel for 8 Trainium2 NeuronCores.

    out = relu(segment_sum(h[col], row) + bias),  h = x @ W

Strategy (dst-block sharding — no collectives needed):
  * Host sorts edges by destination node and buckets them into 157 blocks of
    128 dst nodes; blocks are assigned contiguously to cores (20/core).  Each
    core produces a disjoint slice of the output, so partial aggregates never
    need an all-reduce.
  * Phase A (per core, replicated): h = x @ W on the PE in fp16
    (PSUM fp32 accumulate), streamed to a per-core DRAM buffer h[20096,128]
    fp16.  x is shipped pre-transposed/pre-tiled from the host so each lhsT
    tile is one contiguous 64KB DMA.
  * Phase B: for each dst block, dma_gather (SWDGE) fetches the h rows of the
    block's (padded) edge list into SBUF with edge-on-partition layout
    [128e, PB, 128f]; the DVE builds one-hot tiles S[e,n] = (iota == rowloc)
    in fp16; the PE computes out_block += S^T @ val accumulating all chunks of
    the block in PSUM fp32 — an exact segment-sum.  Bias is folded in as an
    extra "bias chunk" per block (gathers a bias row stored at h[20095] with an
    identity one-hot).  ACT applies ReLU PSUM->SBUF, then the result is DMA'd
    out.

Numerics: fp16 operands with fp32 accumulation everywhere; one-hot matmul is
exact, so the only error is fp16 rounding of x, W and h (~1e-3 relative).
"""

import sys

import numpy as np

sys.path.insert(0, "/opt/trn_rl_repo")

import concourse.bacc as bacc  # noqa: E402
import concourse.bass as bass  # noqa: E402  (engine types)
import concourse.mybir as mybir  # noqa: E402
from concourse.bass_utils import run_bass_kernel_spmd  # noqa: E402

N_NODES = 20000
FIN = 256
FOUT = 128
N_EDGES = 640000

NT = 157                 # node tiles of 128 (nodes padded to 20096)
NPAD = NT * 128          # 20096
NBLK = 157               # dst blocks of 128 nodes
NCORES = 8
NB = 20                  # block slots per core (core 7: 17 real + 3 dummy)
BIAS_ROW = NPAD - 1      # h row that phase-B reads the bias vector from

XT_BUFS = 4              # xT tile ring (phase A)
H_BUFS = 4               # h sbuf tile ring (phase A)
S_BUFS = 4               # one-hot tile ring (phase B)

FP16 = mybir.dt.float16
FP32 = mybir.dt.float32
I16 = mybir.dt.int16


def _host_prep(x, edge_index, weight, bias):
    """Cast/retile operands and bucket edges by destination block."""
    x = np.asarray(x, np.float32)
    weight = np.asarray(weight, np.float32)
    bias = np.asarray(bias, np.float32)

    xpad = np.zeros((NPAD, FIN), np.float32)
    xpad[:N_NODES] = x
    # lhsT tiles: xt_tiles[i, k, kc, n] = x[i*128+n, kc*128+k]
    xt_tiles = np.ascontiguousarray(
        xpad.reshape(NT, 128, 2, 128).transpose(0, 3, 2, 1).astype(np.float16)
    )
    w_t = np.ascontiguousarray(weight.astype(np.float16).reshape(2, 128, 128))
    bias16 = np.ascontiguousarray(bias.astype(np.float16).reshape(1, 128))
    iota16 = np.ascontiguousarray(
        np.broadcast_to(np.arange(128, dtype=np.float16), (128, 128))
    )

    row = np.asarray(edge_index[0]).astype(np.int64)
    col = np.asarray(edge_index[1]).astype(np.int64)
    order = np.argsort(row, kind="stable")
    rs = row[order].astype(np.int32)
    cs = col[order].astype(np.int32)

    blk = rs >> 7
    counts = np.bincount(blk, minlength=NBLK)
    starts = np.concatenate([[0], np.cumsum(counts)])
    pb = int(np.max((counts + 127) // 128)) + 1  # +1 for the bias chunk
    pb = ((pb + 6) // 7) * 7  # sub-gathers of 7 chunks (896 idxs <= SWDGE ring)
    nidx = pb * 128
    idxc = nidx // 16

    col16 = np.zeros((NCORES, 128, NB * idxc), np.int16)
    rloc16 = np.full((NCORES, 128, NB * pb), -1.0, np.float32)
    bias_rl = np.arange(128, dtype=np.float32)
    for c in range(NCORES):
        for s in range(NB):
            b = c * NB + s
            lin_col = np.zeros(nidx, np.int32)
            lin_rl = np.full(nidx, -1.0, np.float32)
            lin_col[:128] = BIAS_ROW          # bias chunk: identity one-hot
            lin_rl[:128] = bias_rl
            if b < NBLK:
                e0, e1 = int(starts[b]), int(starts[b + 1])
                k = e1 - e0
                lin_col[128:128 + k] = cs[e0:e1]
                lin_rl[128:128 + k] = rs[e0:e1] - b * 128
            # the SWDGE tx/rx Q7 pair read the indices from different
            # 16-partition groups — replicate the 16-row wrap to all 128
            col16[c, :, s * idxc:(s + 1) * idxc] = np.tile(
                lin_col.reshape(idxc, 16).T.astype(np.int16), (8, 1)
            )
            rloc16[c, :, s * pb:(s + 1) * pb] = (
                lin_rl.reshape(pb, 128).T.astype(np.float32)
            )
    return xt_tiles, w_t, bias16, iota16, col16, rloc16, pb


def _build_program(pb):
    nidx = pb * 128
    idxc = nidx // 16
    nc = bacc.Bacc("TRN2")

    xt_d = nc.dram_tensor("xt", [NT, 128, 2, 128], FP16, kind="ExternalInput")
    w_d = nc.dram_tensor("w", [2, 128, 128], FP16, kind="ExternalInput")
    b_d = nc.dram_tensor("bias", [1, 128], FP16, kind="ExternalInput")
    io_d = nc.dram_tensor("iota", [128, 128], FP16, kind="ExternalInput")
    col_d = nc.dram_tensor("col", [128, NB * idxc], I16, kind="ExternalInput")
    rl_d = nc.dram_tensor("rl", [128, NB * pb], FP32, kind="ExternalInput")
    h_d = nc.dram_tensor("hbuf", [NPAD, 128], FP16)
    o_d = nc.dram_tensor("out", [NB * 128, 128], FP32, kind="ExternalOutput")

    from contextlib import ExitStack

    with ExitStack() as es:
        ph0 = es.enter_context(nc.psum_tensor("ph0", [128, 512], FP32))
        ph1 = es.enter_context(nc.psum_tensor("ph1", [128, 512], FP32))
        ph2 = es.enter_context(nc.psum_tensor("ph2", [128, 512], FP32))
        ph3 = es.enter_context(nc.psum_tensor("ph3", [128, 512], FP32))
        pb0 = es.enter_context(nc.psum_tensor("pb0", [128, 512], FP32))
        pb1 = es.enter_context(nc.psum_tensor("pb1", [128, 512], FP32))
        w_sb = es.enter_context(nc.sbuf_tensor("w_sb", [128, 2, 128], FP16))
        xt_sb = es.enter_context(
            nc.sbuf_tensor("xt_sb", [128, XT_BUFS, 2, 128], FP16)
        )
        h_sb = es.enter_context(nc.sbuf_tensor("h_sb", [128, H_BUFS, 128], FP16))
        iota_sb = es.enter_context(nc.sbuf_tensor("iota_sb", [128, 128], FP16))
        col_sb = es.enter_context(nc.sbuf_tensor("col_sb", [128, NB * idxc], I16))
        rl_sb = es.enter_context(nc.sbuf_tensor("rl_sb", [128, NB * pb], FP32))
        val_sb = es.enter_context(
            nc.sbuf_tensor("val_sb", [128, 2, pb, 128], FP16)
        )
        s_sb = es.enter_context(nc.sbuf_tensor("s_sb", [128, S_BUFS, 128], FP16))
        o_sb = es.enter_context(nc.sbuf_tensor("o_sb", [128, 2, 128], FP32))
        # DMA-completion sems rotate per ring slot (DMA completions on one
        # sem can reorder, so each slot gets its own counter).
        s_ld = [es.enter_context(nc.semaphore(f"s_ld{k}")) for k in range(5)]
        s_xt = [es.enter_context(nc.semaphore(f"s_xt{k}")) for k in range(XT_BUFS)]
        s_hw = [es.enter_context(nc.semaphore(f"s_hw{k}")) for k in range(H_BUFS)]
        s_bw = es.enter_context(nc.semaphore("s_bw"))
        s_gat = [
            es.enter_context(nc.semaphore(f"s_gat{k}"))
            for k in range(2 * (pb // 7))
        ]
        s_ow = [es.enter_context(nc.semaphore(f"s_ow{k}")) for k in range(2)]
        # compute-engine sems increment in program order (no ambiguity)
        s_hmm = es.enter_context(nc.semaphore("s_hmm"))
        s_hcp = es.enter_context(nc.semaphore("s_hcp"))
        s_s = es.enter_context(nc.semaphore("s_s"))
        s_pmm = es.enter_context(nc.semaphore("s_pmm"))
        s_ocp = es.enter_context(nc.semaphore("s_ocp"))
        block = es.enter_context(nc.Block())
        ph = [ph0, ph1, ph2, ph3]
        pbk = [pb0, pb1]

        hw_total = [16 * len(range(k, NT, H_BUFS)) for k in range(H_BUFS)]

        def store_h(sync, j):
            sync.wait_ge(s_hcp, j + 1)
            sync.dma_start(
                h_d[j * 128:(j + 1) * 128, :], h_sb[:, j % H_BUFS, :]
            ).then_inc(s_hw[j % H_BUFS], 16)

        @block.sync
        def _(sync):
            # one-time loads
            sync.dma_start(w_sb[:, 0, :], w_d[0]).then_inc(s_ld[0], 16)
            sync.dma_start(w_sb[:, 1, :], w_d[1]).then_inc(s_ld[1], 16)
            sync.dma_start(iota_sb[:, :], io_d[:, :]).then_inc(s_ld[2], 16)
            sync.dma_start(col_sb[:, :], col_d[:, :]).then_inc(s_ld[3], 16)
            sync.dma_start(rl_sb[:, :], rl_d[:, :]).then_inc(s_ld[4], 16)
            # phase A: stream xT tiles in, h tiles out (staggered)
            for i in range(NT):
                if i >= XT_BUFS:
                    sync.wait_ge(s_hmm, i - (XT_BUFS - 1))
                sync.dma_start(xt_sb[:, i % XT_BUFS, :, :], xt_d[i]).then_inc(
                    s_xt[i % XT_BUFS], 16
                )
                if i >= 3:
                    store_h(sync, i - 3)
            for j in range(NT - 3, NT):
                store_h(sync, j)
            # bias row (after ALL h writes are complete — tile 156 covers it)
            for k in range(H_BUFS):
                sync.wait_ge(s_hw[k], hw_total[k])
            sync.dma_start(h_d[BIAS_ROW:BIAS_ROW + 1, :], b_d[0:1, :]).then_inc(
                s_bw, 16
            )
            # phase B: output stores
            for b in range(NB):
                sync.wait_ge(s_ocp, b + 1)
                sync.dma_start(
                    o_d[b * 128:(b + 1) * 128, :], o_sb[:, b % 2, :]
                ).then_inc(s_ow[b % 2], 16)

        @block.gpsimd
        def _(gpsimd):
            gpsimd.wait_ge(s_ld[3], 16)
            for k in range(H_BUFS):
                gpsimd.wait_ge(s_hw[k], hw_total[k])
            gpsimd.wait_ge(s_bw, 16)
            for b in range(NB):
                if b >= 2:
                    gpsimd.wait_ge(s_pmm, (b - 1) * pb)
                for g in range(pb // 7):
                    gpsimd.dma_gather(
                        val_sb[:, b % 2, g * 7:(g + 1) * 7, :],
                        h_d[:, :],
                        col_sb[:, b * idxc + g * 56:b * idxc + (g + 1) * 56],
                        896,
                        896,
                        128,
                    ).then_inc(s_gat[(b % 2) * (pb // 7) + g], 16)

        @block.tensor
        def _(tensor):
            for k in range(2):
                tensor.wait_ge(s_ld[k], 16)
            # phase A: h tile i = xT_i^T @ W  (two K chunks)
            for i in range(NT):
                tensor.wait_ge(s_xt[i % XT_BUFS], 16 * (i // XT_BUFS + 1))
                if i >= XT_BUFS:
                    tensor.wait_ge(s_hcp, i - (XT_BUFS - 1))
                tensor.matmul(
                    ph[i % XT_BUFS][:, 0:128],
                    xt_sb[:, i % XT_BUFS, 0, :],
                    w_sb[:, 0, :],
                    start=True,
                    stop=False,
                )
                tensor.matmul(
                    ph[i % XT_BUFS][:, 0:128],
                    xt_sb[:, i % XT_BUFS, 1, :],
                    w_sb[:, 1, :],
                    start=False,
                    stop=True,
                ).then_inc(s_hmm, 1)
            # phase B: out_block += S_chunk^T @ val_chunk
            for b in range(NB):
                if b >= 2:
                    tensor.wait_ge(s_ocp, b - 1)
                for c in range(pb):
                    j = b * pb + c
                    if c % 7 == 0:
                        tensor.wait_ge(
                            s_gat[(b % 2) * (pb // 7) + c // 7],
                            16 * (b // 2 + 1),
                        )
                    tensor.wait_ge(s_s, j + 1)
                    tensor.matmul(
                        pbk[b % 2][:, 0:128],
                        s_sb[:, j % S_BUFS, :],
                        val_sb[:, b % 2, c, :],
                        start=(c == 0),
                        stop=(c == pb - 1),
                    ).then_inc(s_pmm, 1)

        @block.vector
        def _(vector):
            # phase A: PSUM fp32 -> SBUF fp16
            for i in range(NT):
                vector.wait_ge(s_hmm, i + 1)
                if i >= H_BUFS:
                    vector.wait_ge(s_hw[i % H_BUFS], 16 * (i // H_BUFS))
                vector.tensor_copy(
                    h_sb[:, i % H_BUFS, :], ph[i % XT_BUFS][:, 0:128]
                ).then_inc(s_hcp, 1)
            # phase B: one-hot tiles S[e, n] = (iota[n] == rowloc[e])
            vector.wait_ge(s_ld[2], 16)
            vector.wait_ge(s_ld[4], 16)
            for j in range(NB * pb):
                if j >= S_BUFS:
                    vector.wait_ge(s_pmm, j - (S_BUFS - 1))
                vector.tensor_scalar(
                    s_sb[:, j % S_BUFS, :],
                    iota_sb[:, :],
                    rl_sb[:, j:j + 1],
                    None,
                    mybir.AluOpType.is_equal,
                ).then_inc(s_s, 1)

        @block.scalar
        def _(scalar):
            for b in range(NB):
                scalar.wait_ge(s_pmm, (b + 1) * pb)
                if b >= 2:
                    scalar.wait_ge(s_ow[b % 2], 16 * (b // 2))
                scalar.activation(
                    o_sb[:, b % 2, :],
                    pbk[b % 2][:, 0:128],
                    mybir.ActivationFunctionType.Relu,
                ).then_inc(s_ocp, 1)

    nc.compile()
    return nc


def _run(x, edge_index, weight, bias, trace=False):
    xt_tiles, w_t, bias16, iota16, col16, rloc16, pb = _host_prep(
        x, edge_index, weight, bias
    )
    nc = _build_program(pb)
    in_maps = [
        {
            "xt": xt_tiles,
            "w": w_t,
            "bias": bias16,
            "iota": iota16,
            "col": np.ascontiguousarray(col16[c]),
            "rl": np.ascontiguousarray(rloc16[c]),
        }
        for c in range(NCORES)
    ]
    res = run_bass_kernel_spmd(nc, in_maps, list(range(NCORES)), trace=trace)
    out = np.concatenate([res.results[c]["out"] for c in range(NCORES)], axis=0)
    return np.ascontiguousarray(out[:N_NODES]), res


def kernel(x, edge_index, weight, bias):
    out, _ = _run(x, edge_index, weight, bias, trace=False)
    return out



# revision 7
# speedup vs baseline: 2.4513x; 2.4513x over previous
"""GNN message-passing (graph convolution) kernel for 8 Trainium2 NeuronCores.

    out = relu(segment_sum(h[col], row) + bias),  h = x @ W

Strategy (v2, "aggregate-x-then-matmul"): by linearity,
segment_sum(x@W [col], row) = segment_sum(x[col], row) @ W, so the dense
projection is applied AFTER aggregation (20000x256 aggregate rows instead of
640000 edge rows), and the per-edge work is pure data movement:

  * Host LPT-balances the 20000 nodes into 157 dst blocks of 128 (by degree)
    to minimize the global max edges-per-block (=> minimal static chunk count
    pb); blocks are assigned contiguously to cores (20/core), so each core
    produces a disjoint slice of the output - no collectives.
  * Per block b: SWDGE dma_gather fetches x16[col] rows (512B fp16, straight
    from DRAM - x is never staged through SBUF wholesale) into
    val[128e, pb, 256f]; two sub-gathers per block amortize the fixed SWDGE
    cost while bounding the descriptor-ring footprint.
  * DVE builds one-hot tiles S[e,n] = (iota == rowloc) in fp16; the PE
    accumulates aggT[f,n] += val[:,c,f]^T @ S over all chunks of the block in
    PSUM fp32 - an exact transposed segment-sum (no PE transposes needed).
  * DVE copies aggT to SBUF fp16; PE then computes
    out_b = aggT^T @ W + bias (bias via an identity-matmul against a
    broadcast bias tile - no bias gather); ACT applies ReLU; DMA out.

Numerics: fp16 operands with fp32 accumulation; one-hot matmuls are exact, so
the only error is fp16 rounding of x, W and the aggregate (~1e-3 relative).
"""

import sys

import numpy as np

sys.path.insert(0, "/opt/trn_rl_repo")

import concourse.bacc as bacc  # noqa: E402
import concourse.bass as bass  # noqa: E402  (engine types)
import concourse.mybir as mybir  # noqa: E402
from concourse.bass_utils import run_bass_kernel_spmd  # noqa: E402

N_NODES = 20000
FIN = 256
FOUT = 128
N_EDGES = 640000

NBLK = 157               # dst blocks of 128 nodes (157*128 = 20096 slots)
NCORES = 8
NB = 20                  # block slots per core (core 7: 17 real + 3 dummy)

S_BUFS = 4               # one-hot tile ring
DMA_SCRATCH = 16384      # SWDGE ring bytes (ring = DMA_SCRATCH/16 descriptors)
GCH = 7                  # chunks per sub-gather (GCH*128 descriptors each)

FP16 = mybir.dt.float16
FP32 = mybir.dt.float32
I16 = mybir.dt.int16


def _host_prep(x, edge_index, weight, bias):
    """Cast operands, balance nodes into blocks, bucket edges by block."""
    import heapq

    x16 = np.ascontiguousarray(np.asarray(x, np.float32).astype(np.float16))
    weight = np.asarray(weight, np.float32)
    bias = np.asarray(bias, np.float32)

    w_t = np.ascontiguousarray(weight.astype(np.float16).reshape(2, 128, 128))
    bias_bc = np.ascontiguousarray(
        np.broadcast_to(bias.astype(np.float16), (128, 128))
    )
    ident = np.eye(128, dtype=np.float16)
    iota16 = np.ascontiguousarray(
        np.broadcast_to(np.arange(128, dtype=np.float16), (128, 128))
    )

    row = np.asarray(edge_index[0]).astype(np.int64)
    col = np.asarray(edge_index[1]).astype(np.int64)

    # LPT-balance nodes into NBLK blocks of <=128 nodes to minimize the max
    # edges-per-block (sets the global static chunk count pb).
    deg = np.bincount(row, minlength=N_NODES)
    order = np.argsort(-deg, kind="stable")
    blk_of = np.empty(N_NODES, np.int32)
    slot_of = np.empty(N_NODES, np.int32)
    heap = [(0, b) for b in range(NBLK)]
    heapq.heapify(heap)
    nslots = np.zeros(NBLK, np.int32)
    for n in order:
        load, b = heapq.heappop(heap)
        blk_of[n] = b
        slot_of[n] = nslots[b]
        nslots[b] += 1
        if nslots[b] < 128:
            heapq.heappush(heap, (load + int(deg[n]), b))

    b_of_edge = blk_of[row]
    eorder = np.argsort(b_of_edge, kind="stable")
    cs = col[eorder].astype(np.int32)
    rloc = slot_of[row[eorder]].astype(np.float32)
    counts = np.bincount(b_of_edge, minlength=NBLK)
    starts = np.concatenate([[0], np.cumsum(counts)])
    pb = max(2, int(np.max((counts + 127) // 128)))
    nidx = pb * 128
    idxc = nidx // 16

    col16 = np.zeros((NCORES, 128, NB * idxc), np.int16)
    rloc16 = np.full((NCORES, 128, NB * pb), -1.0, np.float32)
    for c in range(NCORES):
        for s in range(NB):
            b = c * NB + s
            lin_col = np.zeros(nidx, np.int32)
            lin_rl = np.full(nidx, -1.0, np.float32)
            if b < NBLK:
                e0, e1 = int(starts[b]), int(starts[b + 1])
                k = e1 - e0
                lin_col[:k] = cs[e0:e1]
                lin_rl[:k] = rloc[e0:e1]
            # the SWDGE tx/rx Q7 pair read the indices from different
            # 16-partition groups - replicate the 16-row wrap to all 128
            col16[c, :, s * idxc:(s + 1) * idxc] = np.tile(
                lin_col.reshape(idxc, 16).T.astype(np.int16), (8, 1)
            )
            rloc16[c, :, s * pb:(s + 1) * pb] = (
                lin_rl.reshape(pb, 128).T.astype(np.float32)
            )
    # out_concat[blk*128 + slot] -> node
    pos = (blk_of * 128 + slot_of).astype(np.int64)
    return x16, w_t, bias_bc, ident, iota16, col16, rloc16, pb, pos


def _build_program(pb):
    nidx = pb * 128
    idxc = nidx // 16
    # sub-gather chunk ranges: groups of GCH chunks (last group smaller)
    gbounds = list(range(0, pb, GCH)) + [pb]
    ngat = len(gbounds) - 1
    nc = bacc.Bacc("TRN2", dynamic_dma_scratch_size=DMA_SCRATCH)

    x_d = nc.dram_tensor("x16", [N_NODES, FIN], FP16, kind="ExternalInput")
    w_d = nc.dram_tensor("w", [2, 128, 128], FP16, kind="ExternalInput")
    bb_d = nc.dram_tensor("bb", [128, 128], FP16, kind="ExternalInput")
    id_d = nc.dram_tensor("ident", [128, 128], FP16, kind="ExternalInput")
    io_d = nc.dram_tensor("iota", [128, 128], FP16, kind="ExternalInput")
    col_d = nc.dram_tensor("col", [128, NB * idxc], I16, kind="ExternalInput")
    rl_d = nc.dram_tensor("rl", [128, NB * pb], FP32, kind="ExternalInput")
    o_d = nc.dram_tensor("out", [NB * 128, 128], FP32, kind="ExternalOutput")

    from contextlib import ExitStack

    with ExitStack() as es:
        # aggT accumulators: [parity][feature-half], one bank each
        pa = [
            [es.enter_context(nc.psum_tensor(f"pa{k}{h}", [128, 512], FP32))
             for h in range(2)]
            for k in range(2)
        ]
        po = [es.enter_context(nc.psum_tensor(f"po{k}", [128, 512], FP32))
              for k in range(2)]
        w_sb = es.enter_context(nc.sbuf_tensor("w_sb", [128, 2, 128], FP16))
        bb_sb = es.enter_context(nc.sbuf_tensor("bb_sb", [128, 128], FP16))
        id_sb = es.enter_context(nc.sbuf_tensor("id_sb", [128, 128], FP16))
        iota_sb = es.enter_context(nc.sbuf_tensor("iota_sb", [128, 128], FP16))
        col_sb = es.enter_context(nc.sbuf_tensor("col_sb", [128, NB * idxc], I16))
        rl_sb = es.enter_context(nc.sbuf_tensor("rl_sb", [128, NB * pb], FP32))
        val_sb = es.enter_context(
            nc.sbuf_tensor("val_sb", [128, 2, pb, FIN], FP16)
        )
        s_sb = es.enter_context(nc.sbuf_tensor("s_sb", [128, S_BUFS, 128], FP16))
        at_sb = es.enter_context(nc.sbuf_tensor("at_sb", [128, 2, 2, 128], FP16))
        o_sb = es.enter_context(nc.sbuf_tensor("o_sb", [128, 2, 128], FP32))

        s_ld = [es.enter_context(nc.semaphore(f"s_ld{k}")) for k in range(7)]
        s_gat = [
            es.enter_context(nc.semaphore(f"s_gat{k}")) for k in range(2 * ngat)
        ]
        s_ow = [es.enter_context(nc.semaphore(f"s_ow{k}")) for k in range(2)]
        s_s = es.enter_context(nc.semaphore("s_s"))      # DVE one-hot count
        s_pmm = es.enter_context(nc.semaphore("s_pmm"))  # PE chunk-mm count
        s_vcp = es.enter_context(nc.semaphore("s_vcp"))  # DVE aggT copies
        s_omm = es.enter_context(nc.semaphore("s_omm"))  # PE final-mm count
        s_ocp = es.enter_context(nc.semaphore("s_ocp"))  # ACT relu count
        block = es.enter_context(nc.Block())

        LD_W, LD_BB, LD_ID, LD_IO, LD_COL, LD_RL = range(6)

        @block.sync
        def _(sync):
            sync.dma_start(w_sb[:, 0, :], w_d[0]).then_inc(s_ld[LD_W], 16)
            sync.dma_start(w_sb[:, 1, :], w_d[1]).then_inc(s_ld[6], 16)
            sync.dma_start(bb_sb[:, :], bb_d[:, :]).then_inc(s_ld[LD_BB], 16)
            sync.dma_start(id_sb[:, :], id_d[:, :]).then_inc(s_ld[LD_ID], 16)
            sync.dma_start(iota_sb[:, :], io_d[:, :]).then_inc(s_ld[LD_IO], 16)
            sync.dma_start(col_sb[:, :], col_d[:, :]).then_inc(s_ld[LD_COL], 16)
            sync.dma_start(rl_sb[:, :], rl_d[:, :]).then_inc(s_ld[LD_RL], 16)
            for b in range(NB):
                sync.wait_ge(s_ocp, b + 1)
                sync.dma_start(
                    o_d[b * 128:(b + 1) * 128, :], o_sb[:, b % 2, :]
                ).then_inc(s_ow[b % 2], 16)

        @block.gpsimd
        def _(gpsimd):
            gpsimd.wait_ge(s_ld[LD_COL], 16)
            for b in range(NB):
                if b >= 2:
                    # val[b%2] fully consumed by chunks of block b-2
                    gpsimd.wait_ge(s_pmm, (b - 1) * pb)
                c0 = b * idxc
                for g in range(ngat):
                    lo, hi = gbounds[g], gbounds[g + 1]
                    gpsimd.dma_gather(
                        val_sb[:, b % 2, lo:hi, :],
                        x_d[:, :],
                        col_sb[:, c0 + lo * 8:c0 + hi * 8],
                        (hi - lo) * 128,
                        (hi - lo) * 128,
                        FIN,
                    ).then_inc(s_gat[(b % 2) * ngat + g], 16)

        def pe_final(tensor, q):
            tensor.wait_ge(s_vcp, 2 * (q + 1))
            if q >= 2:
                tensor.wait_ge(s_ocp, q - 1)
            tensor.matmul(
                po[q % 2][:, 0:128], id_sb[:, :], bb_sb[:, :],
                start=True, stop=False,
            )
            tensor.matmul(
                po[q % 2][:, 0:128], at_sb[:, q % 2, 0, :], w_sb[:, 0, :],
                start=False, stop=False,
            )
            tensor.matmul(
                po[q % 2][:, 0:128], at_sb[:, q % 2, 1, :], w_sb[:, 1, :],
                start=False, stop=True,
            ).then_inc(s_omm, 1)

        @block.tensor
        def _(tensor):
            for k in (LD_W, 6, LD_BB, LD_ID):
                tensor.wait_ge(s_ld[k], 16)
            for b in range(NB):
                for c in range(pb):
                    j = b * pb + c
                    if c == 0 and b >= 2:
                        # pa[b%2] fully copied out (block b-2)
                        tensor.wait_ge(s_vcp, 2 * (b - 1))
                    if c in gbounds:
                        g = gbounds.index(c)
                        tensor.wait_ge(
                            s_gat[(b % 2) * ngat + g], 16 * (b // 2 + 1)
                        )
                    tensor.wait_ge(s_s, j + 1)
                    tensor.matmul(
                        pa[b % 2][0][:, 0:128],
                        val_sb[:, b % 2, c, 0:128],
                        s_sb[:, j % S_BUFS, :],
                        start=(c == 0),
                        stop=(c == pb - 1),
                    )
                    tensor.matmul(
                        pa[b % 2][1][:, 0:128],
                        val_sb[:, b % 2, c, 128:256],
                        s_sb[:, j % S_BUFS, :],
                        start=(c == 0),
                        stop=(c == pb - 1),
                    ).then_inc(s_pmm, 1)
                if b >= 1:
                    pe_final(tensor, b - 1)
            pe_final(tensor, NB - 1)

        def dve_copies(vector, q):
            vector.wait_ge(s_pmm, (q + 1) * pb)
            if q >= 2:
                # at_sb[q%2] consumed by pe_final(q-2)
                vector.wait_ge(s_omm, q - 1)
            vector.tensor_copy(
                at_sb[:, q % 2, 0, :], pa[q % 2][0][:, 0:128]
            ).then_inc(s_vcp, 1)
            vector.tensor_copy(
                at_sb[:, q % 2, 1, :], pa[q % 2][1][:, 0:128]
            ).then_inc(s_vcp, 1)

        @block.vector
        def _(vector):
            vector.wait_ge(s_ld[LD_IO], 16)
            vector.wait_ge(s_ld[LD_RL], 16)
            for b in range(NB):
                for c in range(pb):
                    j = b * pb + c
                    if j >= S_BUFS:
                        vector.wait_ge(s_pmm, j - (S_BUFS - 1))
                    vector.tensor_scalar(
                        s_sb[:, j % S_BUFS, :],
                        iota_sb[:, :],
                        rl_sb[:, j:j + 1],
                        None,
                        mybir.AluOpType.is_equal,
                    ).then_inc(s_s, 1)
                if b >= 1:
                    dve_copies(vector, b - 1)
            dve_copies(vector, NB - 1)

        @block.scalar
        def _(scalar):
            for q in range(NB):
                scalar.wait_ge(s_omm, q + 1)
                if q >= 2:
                    scalar.wait_ge(s_ow[q % 2], 16 * (q // 2))
                scalar.activation(
                    o_sb[:, q % 2, :],
                    po[q % 2][:, 0:128],
                    mybir.ActivationFunctionType.Relu,
                ).then_inc(s_ocp, 1)

    nc.compile()
    return nc


def _run(x, edge_index, weight, bias, trace=False):
    x16, w_t, bias_bc, ident, iota16, col16, rloc16, pb, pos = _host_prep(
        x, edge_index, weight, bias
    )
    nc = _build_program(pb)
    in_maps = [
        {
            "x16": x16,
            "w": w_t,
            "bb": bias_bc,
            "ident": ident,
            "iota": iota16,
            "col": np.ascontiguousarray(col16[c]),
            "rl": np.ascontiguousarray(rloc16[c]),
        }
        for c in range(NCORES)
    ]
    res = run_bass_kernel_spmd(nc, in_maps, list(range(NCORES)), trace=trace)
    out = np.concatenate([res.results[c]["out"] for c in range(NCORES)], axis=0)
    return np.ascontiguousarray(out[pos]), res


def kernel(x, edge_index, weight, bias):
    out, _ = _run(x, edge_index, weight, bias, trace=False)
    return out


# revision 11
# speedup vs baseline: 2.7330x; 1.1149x over previous
"""GNN message-passing (graph convolution) kernel for 8 Trainium2 NeuronCores.

    out = relu(segment_sum(h[col], row) + bias),  h = x @ W

Strategy (v3, "aggregate-x-then-matmul" + dedup): by linearity,
segment_sum(x@W [col], row) = segment_sum(x[col], row) @ W, so the dense
projection is applied AFTER aggregation (20000x256 aggregate rows instead of
640000 edge rows) and the per-edge work is pure data movement:

  * Host LPT-balances the 20000 nodes into 157 dst blocks of 128 (by degree)
    to minimize the max distinct-sources-per-block (=> minimal static chunk
    count pb); blocks are assigned contiguously to cores (20/core), so each
    core produces a disjoint output slice - no collectives.
  * Within a block each distinct source column is gathered ONCE (block-level
    dedup, ~9% fewer rows).  Sources with k>=2 destinations in the block are
    sorted to the front, and the leading chunks run k one-hot passes (the
    static per-chunk pass count is the global max, so the SPMD instruction
    stream is identical on every core).
  * The gather stream is FLAT across blocks: SWDGE dma_gather instructions of
    7 chunks (896 rows, the ring limit) are issued back to back into a
    63-chunk val ring, crossing block boundaries, which minimizes the count
    of gather instructions (994ns fixed cost each on the GpSimd engine).
    x rows (512B fp16) are gathered straight from DRAM - x is never staged
    through SBUF wholesale.
  * DVE builds one-hot tiles S[e,n] = (iota == rowloc[pass]) in fp16; the PE
    accumulates aggT[f,n] += val[:,c,f]^T @ S over all chunks+passes of the
    block in PSUM fp32 - an exact transposed segment-sum (no PE transposes).
  * DVE copies aggT to SBUF fp16; PE computes out_b = aggT^T @ W + bias
    (bias via an identity-matmul against a broadcast bias tile - no bias
    gather); ACT applies ReLU; DMA out.

Numerics: fp16 operands with fp32 accumulation; one-hot matmuls are exact, so
the only error is fp16 rounding of x, W and the aggregate (~1e-3 relative).
"""

import sys

import numpy as np

sys.path.insert(0, "/opt/trn_rl_repo")

import concourse.bacc as bacc  # noqa: E402
import concourse.bass as bass  # noqa: E402  (engine types)
import concourse.mybir as mybir  # noqa: E402
from concourse.bass_utils import run_bass_kernel_spmd  # noqa: E402

N_NODES = 20000
FIN = 256
FOUT = 128
N_EDGES = 640000

NBLK = 157               # dst blocks of 128 nodes (157*128 = 20096 slots)
NCORES = 8
NB = 20                  # block slots per core (core 7: 17 real + 3 dummy)

S_BUFS = 8               # one-hot tile ring
GRP = 7                  # chunks per gather (896 rows <= 1024-desc SWDGE ring)
VC = 63                  # val ring chunks (multiple of GRP)
NSEM = VC // GRP         # rotating gather-completion semaphores

FP16 = mybir.dt.float16
FP32 = mybir.dt.float32
I16 = mybir.dt.int16


def _host_prep(x, edge_index, weight, bias):
    """Cast operands, balance nodes into blocks, dedup+bucket edges."""
    import heapq

    x16 = np.ascontiguousarray(np.asarray(x, np.float32).astype(np.float16))
    weight = np.asarray(weight, np.float32)
    bias = np.asarray(bias, np.float32)

    w_t = np.ascontiguousarray(weight.astype(np.float16).reshape(2, 128, 128))
    bias_bc = np.ascontiguousarray(
        np.broadcast_to(bias.astype(np.float16), (128, 128))
    )
    ident = np.eye(128, dtype=np.float16)
    iota16 = np.ascontiguousarray(
        np.broadcast_to(np.arange(128, dtype=np.float16), (128, 128))
    )

    row = np.asarray(edge_index[0]).astype(np.int64)
    col = np.asarray(edge_index[1]).astype(np.int64)

    # LPT-balance nodes into NBLK blocks of <=128 nodes (by degree) to
    # minimize the max edges-per-block.
    deg = np.bincount(row, minlength=N_NODES)
    order = np.argsort(-deg, kind="stable")
    blk_of = np.empty(N_NODES, np.int32)
    slot_of = np.empty(N_NODES, np.int32)
    heap = [(0, b) for b in range(NBLK)]
    heapq.heapify(heap)
    nslots = np.zeros(NBLK, np.int32)
    for n in order:
        load, b = heapq.heappop(heap)
        blk_of[n] = b
        slot_of[n] = nslots[b]
        nslots[b] += 1
        if nslots[b] < 128:
            heapq.heappush(heap, (load + int(deg[n]), b))

    b_of_edge = blk_of[row]
    eorder = np.argsort(b_of_edge, kind="stable")
    cs = col[eorder].astype(np.int32)
    rloc = slot_of[row[eorder]].astype(np.int32)
    counts = np.bincount(b_of_edge, minlength=NBLK)
    starts = np.concatenate([[0], np.cumsum(counts)])

    # Per block: group edges by source col; sort groups by dst-count desc so
    # multi-pass chunks cluster at the front.  Collect per-chunk dst-count
    # maxima to fix the global static pass schedule.
    blk_cols = []        # per block: unique cols in layout order
    blk_passes = []      # per block: (chunk, pass, part, rloc) per edge
    nuniq = np.zeros(NBLK, np.int32)
    for b in range(NBLK):
        e0, e1 = int(starts[b]), int(starts[b + 1])
        c_b, r_b = cs[e0:e1], rloc[e0:e1]
        o2 = np.argsort(c_b, kind="stable")
        sc, sr = c_b[o2], r_b[o2]
        new = np.r_[True, sc[1:] != sc[:-1]]
        gid = np.cumsum(new) - 1
        first = np.flatnonzero(new)
        dcnt = np.diff(np.r_[first, sc.size])
        rank = np.arange(sc.size) - first[gid]
        uorder = np.argsort(-dcnt, kind="stable")
        pos_of_group = np.empty(dcnt.size, np.int64)
        pos_of_group[uorder] = np.arange(dcnt.size)
        e_pos = pos_of_group[gid]
        blk_cols.append(sc[first[uorder]])
        blk_passes.append((e_pos // 128, rank, e_pos % 128, sr))
        nuniq[b] = dcnt.size

    pb = max(2, int(np.max((nuniq + 127) // 128)))
    dmax = np.zeros(pb, np.int64)
    for b in range(NBLK):
        chunk, rank, part, sr = blk_passes[b]
        np.maximum.at(dmax, chunk, rank + 1)
    passes = np.maximum(dmax, 1)
    pcum = np.concatenate([[0], np.cumsum(passes)])
    pbs = int(pcum[-1])

    nidx = pb * 128
    idxc = nidx // 16
    col16 = np.zeros((NCORES, 128, NB * idxc), np.int16)
    rloc16 = np.full((NCORES, 128, NB * pbs), -1.0, np.float32)
    for c in range(NCORES):
        for s in range(NB):
            b = c * NB + s
            lin_col = np.zeros(nidx, np.int32)
            lin_rl = np.full((pbs, 128), -1.0, np.float32)
            if b < NBLK:
                u = blk_cols[b]
                lin_col[:u.size] = u
                chunk, rank, part, sr = blk_passes[b]
                lin_rl[pcum[chunk] + rank, part] = sr
            # the SWDGE tx/rx Q7 pair read the indices from different
            # 16-partition groups - replicate the 16-row wrap to all 128
            col16[c, :, s * idxc:(s + 1) * idxc] = np.tile(
                lin_col.reshape(idxc, 16).T.astype(np.int16), (8, 1)
            )
            rloc16[c, :, s * pbs:(s + 1) * pbs] = lin_rl.T
    # out_concat[blk*128 + slot] -> node
    pos = (blk_of * 128 + slot_of).astype(np.int64)
    meta = (pb, [int(v) for v in passes])
    return x16, w_t, bias_bc, ident, iota16, col16, rloc16, meta, pos


def _build_program(meta):
    pb, passes = meta
    pcum = [0]
    for v in passes:
        pcum.append(pcum[-1] + v)
    pbs = pcum[-1]
    idxc = pb * 8
    nch = NB * pb                    # global chunk count
    ngat = (nch + GRP - 1) // GRP    # flat gather instructions

    def cum_p(j):  # passes in global chunks [0, j)
        return (j // pb) * pbs + pcum[j % pb]

    nc = bacc.Bacc("TRN2")

    x_d = nc.dram_tensor("x16", [N_NODES, FIN], FP16, kind="ExternalInput")
    w_d = nc.dram_tensor("w", [2, 128, 128], FP16, kind="ExternalInput")
    bb_d = nc.dram_tensor("bb", [128, 128], FP16, kind="ExternalInput")
    id_d = nc.dram_tensor("ident", [128, 128], FP16, kind="ExternalInput")
    io_d = nc.dram_tensor("iota", [128, 128], FP16, kind="ExternalInput")
    col_d = nc.dram_tensor("col", [128, NB * idxc], I16, kind="ExternalInput")
    rl_d = nc.dram_tensor("rl", [128, NB * pbs], FP32, kind="ExternalInput")
    o_d = nc.dram_tensor("out", [NB * 128, 128], FP32, kind="ExternalOutput")

    from contextlib import ExitStack

    with ExitStack() as es:
        # aggT accumulators: [parity][feature-half], one bank each
        pa = [
            [es.enter_context(nc.psum_tensor(f"pa{k}{h}", [128, 512], FP32))
             for h in range(2)]
            for k in range(2)
        ]
        po = [es.enter_context(nc.psum_tensor(f"po{k}", [128, 512], FP32))
              for k in range(2)]
        w_sb = es.enter_context(nc.sbuf_tensor("w_sb", [128, 2, 128], FP16))
        bb_sb = es.enter_context(nc.sbuf_tensor("bb_sb", [128, 128], FP16))
        id_sb = es.enter_context(nc.sbuf_tensor("id_sb", [128, 128], FP16))
        iota_sb = es.enter_context(nc.sbuf_tensor("iota_sb", [128, 128], FP16))
        col_sb = es.enter_context(nc.sbuf_tensor("col_sb", [128, NB * idxc], I16))
        rl_sb = es.enter_context(nc.sbuf_tensor("rl_sb", [128, NB * pbs], FP32))
        val_sb = es.enter_context(nc.sbuf_tensor("val_sb", [128, VC, FIN], FP16))
        s_sb = es.enter_context(nc.sbuf_tensor("s_sb", [128, S_BUFS, 128], FP16))
        at_sb = es.enter_context(nc.sbuf_tensor("at_sb", [128, 2, 2, 128], FP16))
        o_sb = es.enter_context(nc.sbuf_tensor("o_sb", [128, 2, 128], FP32))

        s_ld = [es.enter_context(nc.semaphore(f"s_ld{k}")) for k in range(7)]
        s_gat = [
            es.enter_context(nc.semaphore(f"s_gat{k}")) for k in range(NSEM)
        ]
        s_ow = [es.enter_context(nc.semaphore(f"s_ow{k}")) for k in range(2)]
        s_s = es.enter_context(nc.semaphore("s_s"))      # DVE one-hot count
        s_smm = es.enter_context(nc.semaphore("s_smm"))  # PE pass-mm count
        s_vcp = es.enter_context(nc.semaphore("s_vcp"))  # DVE aggT copies
        s_omm = es.enter_context(nc.semaphore("s_omm"))  # PE final-mm count
        s_ocp = es.enter_context(nc.semaphore("s_ocp"))  # ACT relu count
        block = es.enter_context(nc.Block())

        LD_W, LD_W1, LD_BB, LD_ID, LD_IO, LD_COL, LD_RL = range(7)

        @block.sync
        def _(sync):
            sync.dma_start(w_sb[:, 0, :], w_d[0]).then_inc(s_ld[LD_W], 16)
            sync.dma_start(w_sb[:, 1, :], w_d[1]).then_inc(s_ld[LD_W1], 16)
            sync.dma_start(bb_sb[:, :], bb_d[:, :]).then_inc(s_ld[LD_BB], 16)
            sync.dma_start(id_sb[:, :], id_d[:, :]).then_inc(s_ld[LD_ID], 16)
            sync.dma_start(iota_sb[:, :], io_d[:, :]).then_inc(s_ld[LD_IO], 16)
            sync.dma_start(col_sb[:, :], col_d[:, :]).then_inc(s_ld[LD_COL], 16)
            sync.dma_start(rl_sb[:, :], rl_d[:, :]).then_inc(s_ld[LD_RL], 16)
            for b in range(NB):
                sync.wait_ge(s_ocp, b + 1)
                sync.dma_start(
                    o_d[b * 128:(b + 1) * 128, :], o_sb[:, b % 2, :]
                ).then_inc(s_ow[b % 2], 16)

        @block.gpsimd
        def _(gpsimd):
            gpsimd.wait_ge(s_ld[LD_COL], 16)
            for g in range(ngat):
                j0, j1 = GRP * g, min(GRP * g + GRP, nch)
                if j1 - VC > 0:
                    # val ring slots [j0%VC, ...) held chunks [j0-VC, j1-VC)
                    gpsimd.wait_ge(s_smm, cum_p(j1 - VC))
                r = j0 % VC
                gpsimd.dma_gather(
                    val_sb[:, r:r + (j1 - j0), :],
                    x_d[:, :],
                    col_sb[:, j0 * 8:j1 * 8],
                    (j1 - j0) * 128,
                    (j1 - j0) * 128,
                    FIN,
                ).then_inc(s_gat[g % NSEM], 16)

        def pe_final(tensor, q):
            tensor.wait_ge(s_vcp, 2 * (q + 1))
            if q >= 2:
                tensor.wait_ge(s_ocp, q - 1)
            tensor.matmul(
                po[q % 2][:, 0:128], id_sb[:, :], bb_sb[:, :],
                start=True, stop=False,
            )
            tensor.matmul(
                po[q % 2][:, 0:128], at_sb[:, q % 2, 0, :], w_sb[:, 0, :],
                start=False, stop=False,
            )
            tensor.matmul(
                po[q % 2][:, 0:128], at_sb[:, q % 2, 1, :], w_sb[:, 1, :],
                start=False, stop=True,
            ).then_inc(s_omm, 1)

        @block.tensor
        def _(tensor):
            for k in (LD_W, LD_W1, LD_BB, LD_ID):
                tensor.wait_ge(s_ld[k], 16)
            kk = 0
            for b in range(NB):
                for c in range(pb):
                    j = b * pb + c
                    if j % GRP == 0:
                        g = j // GRP
                        tensor.wait_ge(s_gat[g % NSEM], 16 * (g // NSEM + 1))
                    if c == 0 and b >= 2:
                        # pa[b%2] fully copied out (block b-2)
                        tensor.wait_ge(s_vcp, 2 * (b - 1))
                    for p in range(passes[c]):
                        tensor.wait_ge(s_s, kk + 1)
                        st = c == 0 and p == 0
                        sp = c == pb - 1 and p == passes[c] - 1
                        tensor.matmul(
                            pa[b % 2][0][:, 0:128],
                            val_sb[:, j % VC, 0:128],
                            s_sb[:, kk % S_BUFS, :],
                            start=st,
                            stop=sp,
                        )
                        tensor.matmul(
                            pa[b % 2][1][:, 0:128],
                            val_sb[:, j % VC, 128:256],
                            s_sb[:, kk % S_BUFS, :],
                            start=st,
                            stop=sp,
                        ).then_inc(s_smm, 1)
                        kk += 1
                if b >= 1:
                    pe_final(tensor, b - 1)
            pe_final(tensor, NB - 1)

        def dve_copies(vector, q):
            vector.wait_ge(s_smm, (q + 1) * pbs)
            if q >= 2:
                # at_sb[q%2] consumed by pe_final(q-2)
                vector.wait_ge(s_omm, q - 1)
            vector.tensor_copy(
                at_sb[:, q % 2, 0, :], pa[q % 2][0][:, 0:128]
            ).then_inc(s_vcp, 1)
            vector.tensor_copy(
                at_sb[:, q % 2, 1, :], pa[q % 2][1][:, 0:128]
            ).then_inc(s_vcp, 1)

        @block.vector
        def _(vector):
            vector.wait_ge(s_ld[LD_IO], 16)
            vector.wait_ge(s_ld[LD_RL], 16)
            kk = 0
            for b in range(NB):
                for c in range(pb):
                    for p in range(passes[c]):
                        if kk >= S_BUFS:
                            vector.wait_ge(s_smm, kk - S_BUFS + 1)
                        slot = b * pbs + pcum[c] + p
                        vector.tensor_scalar(
                            s_sb[:, kk % S_BUFS, :],
                            iota_sb[:, :],
                            rl_sb[:, slot:slot + 1],
                            None,
                            mybir.AluOpType.is_equal,
                        ).then_inc(s_s, 1)
                        kk += 1
                if b >= 1:
                    dve_copies(vector, b - 1)
            dve_copies(vector, NB - 1)

        @block.scalar
        def _(scalar):
            for q in range(NB):
                scalar.wait_ge(s_omm, q + 1)
                if q >= 2:
                    scalar.wait_ge(s_ow[q % 2], 16 * (q // 2))
                scalar.activation(
                    o_sb[:, q % 2, :],
                    po[q % 2][:, 0:128],
                    mybir.ActivationFunctionType.Relu,
                ).then_inc(s_ocp, 1)

    nc.compile()
    return nc


def _run(x, edge_index, weight, bias, trace=False):
    x16, w_t, bias_bc, ident, iota16, col16, rloc16, meta, pos = _host_prep(
        x, edge_index, weight, bias
    )
    nc = _build_program(meta)
    in_maps = [
        {
            "x16": x16,
            "w": w_t,
            "bb": bias_bc,
            "ident": ident,
            "iota": iota16,
            "col": np.ascontiguousarray(col16[c]),
            "rl": np.ascontiguousarray(rloc16[c]),
        }
        for c in range(NCORES)
    ]
    res = run_bass_kernel_spmd(nc, in_maps, list(range(NCORES)), trace=trace)
    out = np.concatenate([res.results[c]["out"] for c in range(NCORES)], axis=0)
    return np.ascontiguousarray(out[pos]), res


def kernel(x, edge_index, weight, bias):
    out, _ = _run(x, edge_index, weight, bias, trace=False)
    return out


# revision 13
# speedup vs baseline: 3.0152x; 1.1033x over previous
"""GNN message-passing (graph convolution) kernel for 8 Trainium2 NeuronCores.

    out = relu(segment_sum(h[col], row) + bias),  h = x @ W

Strategy (v4, "aggregate-x-then-matmul" + paired-block dedup): by linearity,
segment_sum(x@W [col], row) = segment_sum(x[col], row) @ W, so the dense
projection is applied AFTER aggregation and the per-edge work is pure data
movement:

  * Host LPT-balances the 20000 nodes into 157 dst blocks of 128 (by degree);
    blocks are assigned contiguously to cores (20/core), so each core
    produces a disjoint output slice - no collectives.
  * Blocks are processed in PAIRS (A,B) sharing one gather stream laid out
    [A-only | A-and-B | B-only]: each distinct source column of the pair is
    gathered ONCE (block-level dedup plus pair-level sharing, ~17% fewer
    rows than raw edges).  A's one-hot matmul range covers the first part of
    the stream, B's the last; the shared middle is consumed by both.
    Sources with k>=2 destinations inside one block run k one-hot passes and
    are sorted to the front of their region (the static per-chunk pass count
    is the global max, so the SPMD instruction stream is identical on every
    core).
  * The gather stream is FLAT: SWDGE dma_gather instructions of 7 chunks
    (896 rows, the ring limit) are issued back to back into an 84-chunk val
    ring, crossing pair boundaries, minimizing the count of gather
    instructions (994ns fixed cost each on the GpSimd engine).  x rows
    (512B fp16) are gathered straight from DRAM.
  * DVE builds one-hot tiles S[e,n] = (iota == rowloc[pass]) in fp16; the PE
    accumulates aggT[f,n] += val[:,c,f]^T @ S over the block's range in PSUM
    fp32 - an exact transposed segment-sum (no PE transposes).
  * DVE copies aggT to SBUF fp16; PE computes out_b = aggT^T @ W + bias
    (bias via an identity-matmul against a broadcast bias tile); ACT applies
    ReLU (fp16 out); DMA out; host re-permutes and upcasts.

Numerics: fp16 operands with fp32 accumulation; one-hot matmuls are exact, so
the only error is fp16 rounding of x, W and the aggregate (~1e-3 relative).
"""

import sys

import numpy as np

sys.path.insert(0, "/opt/trn_rl_repo")

import concourse.bacc as bacc  # noqa: E402
import concourse.bass as bass  # noqa: E402  (engine types)
import concourse.mybir as mybir  # noqa: E402
from concourse.bass_utils import run_bass_kernel_spmd  # noqa: E402

N_NODES = 20000
FIN = 256
FOUT = 128
N_EDGES = 640000

NBLK = 157               # dst blocks of 128 nodes (157*128 = 20096 slots)
NCORES = 8
NB = 20                  # block slots per core (core 7: 17 real + 3 dummy)
NPAIR = NB // 2

S_BUFS = 8               # one-hot tile ring
GRP = 7                  # chunks per gather (896 rows <= 1024-desc SWDGE ring)
VC = 84                  # val ring chunks (multiple of GRP)
NSEM = VC // GRP         # rotating gather-completion semaphores

FP16 = mybir.dt.float16
FP32 = mybir.dt.float32
I16 = mybir.dt.int16


def _host_prep(x, edge_index, weight, bias):
    """Cast operands, balance nodes into blocks, build paired gather streams."""
    import heapq

    x16 = np.ascontiguousarray(np.asarray(x, np.float32).astype(np.float16))
    weight = np.asarray(weight, np.float32)
    bias = np.asarray(bias, np.float32)

    w_t = np.ascontiguousarray(weight.astype(np.float16).reshape(2, 128, 128))
    bias_bc = np.ascontiguousarray(
        np.broadcast_to(bias.astype(np.float16), (128, 128))
    )
    ident = np.eye(128, dtype=np.float16)
    iota16 = np.ascontiguousarray(
        np.broadcast_to(np.arange(128, dtype=np.float16), (128, 128))
    )

    row = np.asarray(edge_index[0]).astype(np.int64)
    col = np.asarray(edge_index[1]).astype(np.int64)

    # LPT-balance nodes into NBLK blocks of <=128 nodes (by degree) to
    # minimize the max edges-per-block.
    deg = np.bincount(row, minlength=N_NODES)
    order = np.argsort(-deg, kind="stable")
    blk_of = np.empty(N_NODES, np.int32)
    slot_of = np.empty(N_NODES, np.int32)
    heap = [(0, b) for b in range(NBLK)]
    heapq.heapify(heap)
    nslots = np.zeros(NBLK, np.int32)
    for n in order:
        load, b = heapq.heappop(heap)
        blk_of[n] = b
        slot_of[n] = nslots[b]
        nslots[b] += 1
        if nslots[b] < 128:
            heapq.heappush(heap, (load + int(deg[n]), b))

    b_of_edge = blk_of[row]
    eorder = np.argsort(b_of_edge, kind="stable")
    cs = col[eorder].astype(np.int32)
    rloc = slot_of[row[eorder]].astype(np.int32)
    counts = np.bincount(b_of_edge, minlength=NBLK)
    starts = np.concatenate([[0], np.cumsum(counts)])

    # Build one dedup'd stream per block pair (A = even slot, B = odd slot).
    # pairs[t of core c] = (global block id A, global block id B or -1)
    pair_blocks = []
    for c in range(NCORES):
        ids = list(range(c * NB, min((c + 1) * NB, NBLK)))
        ids += [-1] * (NB - len(ids))
        for t in range(NPAIR):
            pair_blocks.append((ids[2 * t], ids[2 * t + 1]))

    def blk_edges(g):
        if g < 0:
            return (np.empty(0, np.int32), np.empty(0, np.int32))
        e0, e1 = int(starts[g]), int(starts[g + 1])
        return cs[e0:e1], rloc[e0:e1]

    streams = []   # per pair: (cols, eA_pos, eA_rank, eA_rl, eB_...) arrays
    dims = []      # per pair: (n_only_a, n_a, n_stream)
    for gA, gB in pair_blocks:
        cA, rA = blk_edges(gA)
        cB, rB = blk_edges(gB)
        cp = np.concatenate([cA, cB])
        rp = np.concatenate([rA, rB])
        fp = np.concatenate(
            [np.zeros(cA.size, np.int8), np.ones(cB.size, np.int8)]
        )
        if cp.size == 0:
            streams.append((np.empty(0, np.int32),) + (np.empty(0, np.int64),) * 6)
            dims.append((0, 0, 0))
            continue
        o2 = np.argsort(cp, kind="stable")
        sc, sr, sf = cp[o2], rp[o2], fp[o2]
        new = np.r_[True, sc[1:] != sc[:-1]]
        gid = np.cumsum(new) - 1
        first = np.flatnonzero(new)
        ng = first.size
        dA = np.bincount(gid, weights=1 - sf.astype(np.int64), minlength=ng)
        dB = np.bincount(gid, weights=sf.astype(np.int64), minlength=ng)
        dA = dA.astype(np.int64)
        dB = dB.astype(np.int64)
        rank = np.arange(sc.size) - first[gid]       # A entries lead (stable)
        prank = np.where(sf == 0, rank, rank - dA[gid])
        # region 0: A-only, 1: shared, 2: B-only; sort by key desc within
        region = np.where(dB == 0, 0, np.where(dA == 0, 2, 1))
        key = np.where(region == 0, dA, np.where(region == 2, dB, np.maximum(dA, dB)))
        uorder = np.lexsort((-key, region))
        pos_of_group = np.empty(ng, np.int64)
        pos_of_group[uorder] = np.arange(ng)
        e_pos = pos_of_group[gid]
        isA = sf == 0
        streams.append((
            sc[first[uorder]].astype(np.int32),
            e_pos[isA], prank[isA], sr[isA].astype(np.int64),
            e_pos[~isA], prank[~isA], sr[~isA].astype(np.int64),
        ))
        dims.append((
            int((region[uorder] == 0).sum()),
            int((region[uorder] <= 1).sum()),
            ng,
        ))

    only_a = np.array([d[0] for d in dims])
    n_a = np.array([d[1] for d in dims])
    n_s = np.array([d[2] for d in dims])
    ra = int(np.max((n_a + 127) // 128))            # A-range chunks [0, ra)
    sa = int(np.min(only_a[n_s > 0] // 128))        # B-range start chunk
    pc = int(np.max((n_s + 127) // 128))            # chunks per pair stream
    pc = max(pc, ra, sa + 1)

    # Static pass schedules: global per-chunk max of dst-counts.
    pA = np.zeros(ra, np.int64)
    pB = np.zeros(pc - sa, np.int64)
    for (colsu, ap, ar, arl, bp, br, brl), _ in zip(streams, dims):
        if ap.size:
            np.maximum.at(pA, ap // 128, ar + 1)
        if bp.size:
            np.maximum.at(pB, bp // 128 - sa, br + 1)
    passes_a = np.maximum(pA, 1)
    passes_b = np.maximum(pB, 1)
    pcum_a = np.concatenate([[0], np.cumsum(passes_a)])
    pcum_b = np.concatenate([[0], np.cumsum(passes_b)])
    pa_tot, pb_tot = int(pcum_a[-1]), int(pcum_b[-1])
    pbs2 = pa_tot + pb_tot

    nidx = pc * 128
    idxc = nidx // 16
    col16 = np.zeros((NCORES, 128, NPAIR * idxc), np.int16)
    rloc16 = np.full((NCORES, 128, NPAIR * pbs2), -1.0, np.float32)
    for c in range(NCORES):
        for t in range(NPAIR):
            pi = c * NPAIR + t
            colsu, ap, ar, arl, bp, br, brl = streams[pi]
            lin_col = np.zeros(nidx, np.int32)
            lin_col[:colsu.size] = colsu
            lin_rl = np.full((pbs2, 128), -1.0, np.float32)
            if ap.size:
                lin_rl[pcum_a[ap // 128] + ar, ap % 128] = arl
            if bp.size:
                lin_rl[pa_tot + pcum_b[bp // 128 - sa] + br, bp % 128] = brl
            # the SWDGE tx/rx Q7 pair read the indices from different
            # 16-partition groups - replicate the 16-row wrap to all 128
            col16[c, :, t * idxc:(t + 1) * idxc] = np.tile(
                lin_col.reshape(idxc, 16).T.astype(np.int16), (8, 1)
            )
            rloc16[c, :, t * pbs2:(t + 1) * pbs2] = lin_rl.T
    # out_concat[blk*128 + slot] -> node (block ids are already slot order)
    pos = (blk_of * 128 + slot_of).astype(np.int64)
    meta = (pc, sa, ra, [int(v) for v in passes_a], [int(v) for v in passes_b])
    return x16, w_t, bias_bc, ident, iota16, col16, rloc16, meta, pos


def _build_program(meta):
    pc, sa, ra, passes_a, passes_b = meta
    pcum_a = [0]
    for v in passes_a:
        pcum_a.append(pcum_a[-1] + v)
    pcum_b = [0]
    for v in passes_b:
        pcum_b.append(pcum_b[-1] + v)
    pa_tot, pb_tot = pcum_a[-1], pcum_b[-1]
    pbs2 = pa_tot + pb_tot
    idxc = pc * 8
    nch = NPAIR * pc                 # global chunk count
    ngat = (nch + GRP - 1) // GRP    # flat gather instructions

    # range of block-slot q (pair t = q//2): list of (chunk j, passes, rlbase)
    def q_range(q):
        t = q // 2
        if q % 2 == 0:
            return [
                (t * pc + u, passes_a[u], t * pbs2 + pcum_a[u])
                for u in range(ra)
            ]
        return [
            (t * pc + sa + i, passes_b[i], t * pbs2 + pa_tot + pcum_b[i])
            for i in range(pc - sa)
        ]

    def smm_after(q):  # s_smm value after block-slot q's range completes
        t = q // 2
        return t * pbs2 + pa_tot if q % 2 == 0 else (t + 1) * pbs2

    def consume_tgt(j):  # s_smm value at which chunk j is fully consumed
        t, u = j // pc, j % pc
        if u < sa:
            return t * pbs2 + pcum_a[u + 1]
        return t * pbs2 + pa_tot + pcum_b[u - sa + 1]

    nc = bacc.Bacc("TRN2")

    x_d = nc.dram_tensor("x16", [N_NODES, FIN], FP16, kind="ExternalInput")
    w_d = nc.dram_tensor("w", [2, 128, 128], FP16, kind="ExternalInput")
    bb_d = nc.dram_tensor("bb", [128, 128], FP16, kind="ExternalInput")
    id_d = nc.dram_tensor("ident", [128, 128], FP16, kind="ExternalInput")
    io_d = nc.dram_tensor("iota", [128, 128], FP16, kind="ExternalInput")
    col_d = nc.dram_tensor("col", [128, NPAIR * idxc], I16, kind="ExternalInput")
    rl_d = nc.dram_tensor("rl", [128, NPAIR * pbs2], FP32, kind="ExternalInput")
    o_d = nc.dram_tensor("out", [NB * 128, 128], FP16, kind="ExternalOutput")

    from contextlib import ExitStack

    with ExitStack() as es:
        # aggT accumulators: [parity][feature-half], one bank each
        pa = [
            [es.enter_context(nc.psum_tensor(f"pa{k}{h}", [128, 512], FP32))
             for h in range(2)]
            for k in range(2)
        ]
        po = [es.enter_context(nc.psum_tensor(f"po{k}", [128, 512], FP32))
              for k in range(2)]
        w_sb = es.enter_context(nc.sbuf_tensor("w_sb", [128, 2, 128], FP16))
        bb_sb = es.enter_context(nc.sbuf_tensor("bb_sb", [128, 128], FP16))
        id_sb = es.enter_context(nc.sbuf_tensor("id_sb", [128, 128], FP16))
        iota_sb = es.enter_context(nc.sbuf_tensor("iota_sb", [128, 128], FP16))
        col_sb = es.enter_context(
            nc.sbuf_tensor("col_sb", [128, NPAIR * idxc], I16)
        )
        rl_sb = es.enter_context(
            nc.sbuf_tensor("rl_sb", [128, NPAIR * pbs2], FP32)
        )
        val_sb = es.enter_context(nc.sbuf_tensor("val_sb", [128, VC, FIN], FP16))
        s_sb = es.enter_context(nc.sbuf_tensor("s_sb", [128, S_BUFS, 128], FP16))
        at_sb = es.enter_context(nc.sbuf_tensor("at_sb", [128, 2, 2, 128], FP16))
        o_sb = es.enter_context(nc.sbuf_tensor("o_sb", [128, 2, 128], FP16))

        s_ld = [es.enter_context(nc.semaphore(f"s_ld{k}")) for k in range(7)]
        s_gat = [
            es.enter_context(nc.semaphore(f"s_gat{k}")) for k in range(NSEM)
        ]
        s_ow = [es.enter_context(nc.semaphore(f"s_ow{k}")) for k in range(2)]
        s_s = es.enter_context(nc.semaphore("s_s"))      # DVE one-hot count
        s_smm = es.enter_context(nc.semaphore("s_smm"))  # PE pass-mm count
        s_vcp = es.enter_context(nc.semaphore("s_vcp"))  # DVE aggT copies
        s_omm = es.enter_context(nc.semaphore("s_omm"))  # PE final-mm count
        s_ocp = es.enter_context(nc.semaphore("s_ocp"))  # ACT relu count
        block = es.enter_context(nc.Block())

        LD_W, LD_W1, LD_BB, LD_ID, LD_IO, LD_COL, LD_RL = range(7)

        @block.sync
        def _(sync):
            sync.dma_start(w_sb[:, 0, :], w_d[0]).then_inc(s_ld[LD_W], 16)
            sync.dma_start(w_sb[:, 1, :], w_d[1]).then_inc(s_ld[LD_W1], 16)
            sync.dma_start(bb_sb[:, :], bb_d[:, :]).then_inc(s_ld[LD_BB], 16)
            sync.dma_start(id_sb[:, :], id_d[:, :]).then_inc(s_ld[LD_ID], 16)
            sync.dma_start(iota_sb[:, :], io_d[:, :]).then_inc(s_ld[LD_IO], 16)
            sync.dma_start(col_sb[:, :], col_d[:, :]).then_inc(s_ld[LD_COL], 16)
            sync.dma_start(rl_sb[:, :], rl_d[:, :]).then_inc(s_ld[LD_RL], 16)
            for b in range(NB):
                sync.wait_ge(s_ocp, b + 1)
                sync.dma_start(
                    o_d[b * 128:(b + 1) * 128, :], o_sb[:, b % 2, :]
                ).then_inc(s_ow[b % 2], 16)

        @block.gpsimd
        def _(gpsimd):
            gpsimd.wait_ge(s_ld[LD_COL], 16)
            for g in range(ngat):
                j0, j1 = GRP * g, min(GRP * g + GRP, nch)
                if j1 - VC > 0:
                    # val ring slots [j0%VC, ...) held chunks [j0-VC, j1-VC)
                    gpsimd.wait_ge(s_smm, consume_tgt(j1 - VC - 1))
                r = j0 % VC
                gpsimd.dma_gather(
                    val_sb[:, r:r + (j1 - j0), :],
                    x_d[:, :],
                    col_sb[:, j0 * 8:j1 * 8],
                    (j1 - j0) * 128,
                    (j1 - j0) * 128,
                    FIN,
                ).then_inc(s_gat[g % NSEM], 16)

        def pe_final(tensor, q):
            tensor.wait_ge(s_vcp, 2 * (q + 1))
            if q >= 2:
                tensor.wait_ge(s_ocp, q - 1)
            tensor.matmul(
                po[q % 2][:, 0:128], id_sb[:, :], bb_sb[:, :],
                start=True, stop=False,
            )
            tensor.matmul(
                po[q % 2][:, 0:128], at_sb[:, q % 2, 0, :], w_sb[:, 0, :],
                start=False, stop=False,
            )
            tensor.matmul(
                po[q % 2][:, 0:128], at_sb[:, q % 2, 1, :], w_sb[:, 1, :],
                start=False, stop=True,
            ).then_inc(s_omm, 1)

        @block.tensor
        def _(tensor):
            for k in (LD_W, LD_W1, LD_BB, LD_ID):
                tensor.wait_ge(s_ld[k], 16)
            kk = 0
            waited_g = 0
            for q in range(NB):
                rng = q_range(q)
                for i, (j, np_, rlb) in enumerate(rng):
                    g = j // GRP
                    while waited_g <= g:
                        tensor.wait_ge(
                            s_gat[waited_g % NSEM],
                            16 * (waited_g // NSEM + 1),
                        )
                        waited_g += 1
                    if i == 0 and q >= 2:
                        # pa[q%2] fully copied out (block-slot q-2)
                        tensor.wait_ge(s_vcp, 2 * (q - 1))
                    for p in range(np_):
                        tensor.wait_ge(s_s, kk + 1)
                        st = i == 0 and p == 0
                        sp = i == len(rng) - 1 and p == np_ - 1
                        tensor.matmul(
                            pa[q % 2][0][:, 0:128],
                            val_sb[:, j % VC, 0:128],
                            s_sb[:, kk % S_BUFS, :],
                            start=st,
                            stop=sp,
                        )
                        tensor.matmul(
                            pa[q % 2][1][:, 0:128],
                            val_sb[:, j % VC, 128:256],
                            s_sb[:, kk % S_BUFS, :],
                            start=st,
                            stop=sp,
                        ).then_inc(s_smm, 1)
                        kk += 1
                if q >= 1:
                    pe_final(tensor, q - 1)
            pe_final(tensor, NB - 1)

        def dve_copies(vector, q):
            vector.wait_ge(s_smm, smm_after(q))
            if q >= 2:
                # at_sb[q%2] consumed by pe_final(q-2)
                vector.wait_ge(s_omm, q - 1)
            vector.tensor_copy(
                at_sb[:, q % 2, 0, :], pa[q % 2][0][:, 0:128]
            ).then_inc(s_vcp, 1)
            vector.tensor_copy(
                at_sb[:, q % 2, 1, :], pa[q % 2][1][:, 0:128]
            ).then_inc(s_vcp, 1)

        @block.vector
        def _(vector):
            vector.wait_ge(s_ld[LD_IO], 16)
            vector.wait_ge(s_ld[LD_RL], 16)
            kk = 0
            for q in range(NB):
                for i, (j, np_, rlb) in enumerate(q_range(q)):
                    for p in range(np_):
                        if kk >= S_BUFS:
                            vector.wait_ge(s_smm, kk - S_BUFS + 1)
                        slot = rlb + p
                        vector.tensor_scalar(
                            s_sb[:, kk % S_BUFS, :],
                            iota_sb[:, :],
                            rl_sb[:, slot:slot + 1],
                            None,
                            mybir.AluOpType.is_equal,
                        ).then_inc(s_s, 1)
                        kk += 1
                if q >= 1:
                    dve_copies(vector, q - 1)
            dve_copies(vector, NB - 1)

        @block.scalar
        def _(scalar):
            for q in range(NB):
                scalar.wait_ge(s_omm, q + 1)
                if q >= 2:
                    scalar.wait_ge(s_ow[q % 2], 16 * (q // 2))
                scalar.activation(
                    o_sb[:, q % 2, :],
                    po[q % 2][:, 0:128],
                    mybir.ActivationFunctionType.Relu,
                ).then_inc(s_ocp, 1)

    nc.compile()
    return nc


def _run(x, edge_index, weight, bias, trace=False):
    x16, w_t, bias_bc, ident, iota16, col16, rloc16, meta, pos = _host_prep(
        x, edge_index, weight, bias
    )
    nc = _build_program(meta)
    in_maps = [
        {
            "x16": x16,
            "w": w_t,
            "bb": bias_bc,
            "ident": ident,
            "iota": iota16,
            "col": np.ascontiguousarray(col16[c]),
            "rl": np.ascontiguousarray(rloc16[c]),
        }
        for c in range(NCORES)
    ]
    res = run_bass_kernel_spmd(nc, in_maps, list(range(NCORES)), trace=trace)
    out = np.concatenate([res.results[c]["out"] for c in range(NCORES)], axis=0)
    return np.ascontiguousarray(out[pos].astype(np.float32)), res


def kernel(x, edge_index, weight, bias):
    out, _ = _run(x, edge_index, weight, bias, trace=False)
    return out


# revision 19
# speedup vs baseline: 3.0978x; 1.0274x over previous
"""GNN message-passing (graph convolution) kernel for 8 Trainium2 NeuronCores.

    out = relu(segment_sum(h[col], row) + bias),  h = x @ W

Strategy (v4, "aggregate-x-then-matmul" + paired-block dedup): by linearity,
segment_sum(x@W [col], row) = segment_sum(x[col], row) @ W, so the dense
projection is applied AFTER aggregation and the per-edge work is pure data
movement:

  * Host LPT-balances the 20000 nodes into 157 dst blocks of 128 (by degree);
    blocks are assigned contiguously to cores (20/core), so each core
    produces a disjoint output slice - no collectives.
  * Blocks are processed in PAIRS (A,B) sharing one gather stream laid out
    [A-only | A-and-B | B-only]: each distinct source column of the pair is
    gathered ONCE (block-level dedup plus pair-level sharing, ~17% fewer
    rows than raw edges).  A's one-hot matmul range covers the first part of
    the stream, B's the last; the shared middle is consumed by both.
    Sources with k>=2 destinations inside one block run k one-hot passes and
    are sorted to the front of their region (the static per-chunk pass count
    is the global max, so the SPMD instruction stream is identical on every
    core).
  * The gather stream is FLAT: SWDGE dma_gather instructions of 7 chunks
    (896 rows, the ring limit) are issued back to back into an 84-chunk val
    ring, crossing pair boundaries, minimizing the count of gather
    instructions (994ns fixed cost each on the GpSimd engine).  x rows
    (512B fp16) are gathered straight from DRAM.
  * DVE builds one-hot tiles S[e,n] = (iota == rowloc[pass]) in fp16; the PE
    accumulates aggT[f,n] += val[:,c,f]^T @ S over the block's range in PSUM
    fp32 - an exact transposed segment-sum (no PE transposes).
  * DVE copies aggT to SBUF fp16; PE computes out_b = aggT^T @ W + bias
    (bias via an identity-matmul against a broadcast bias tile); ACT applies
    ReLU (fp16 out); DMA out; host re-permutes and upcasts.

Numerics: fp16 operands with fp32 accumulation; one-hot matmuls are exact, so
the only error is fp16 rounding of x, W and the aggregate (~1e-3 relative).
"""

import sys

import numpy as np

sys.path.insert(0, "/opt/trn_rl_repo")

import concourse.bacc as bacc  # noqa: E402
import concourse.bass as bass  # noqa: E402  (engine types)
import concourse.mybir as mybir  # noqa: E402
from concourse.bass_utils import run_bass_kernel_spmd  # noqa: E402

N_NODES = 20000
FIN = 256
FOUT = 128
N_EDGES = 640000

NBLK = 157               # dst blocks of 128 nodes (157*128 = 20096 slots)
NCORES = 8
NB = 20                  # block slots per core (core 7: 17 real + 3 dummy)
NPAIR = NB // 2

S_BUFS = 8               # one-hot tile ring
GRP = 7                  # chunks per gather (896 rows <= 1024-desc SWDGE ring)
VC = 84                  # val ring chunks (multiple of GRP)
NSEM = VC // GRP         # rotating gather-completion semaphores

FP16 = mybir.dt.float16
FP32 = mybir.dt.float32
I16 = mybir.dt.int16


def _host_prep(x, edge_index, weight, bias):
    """Cast operands, balance nodes into blocks, build paired gather streams."""
    import heapq

    x16 = np.ascontiguousarray(np.asarray(x, np.float32).astype(np.float16))
    weight = np.asarray(weight, np.float32)
    bias = np.asarray(bias, np.float32)

    w_t = np.ascontiguousarray(weight.astype(np.float16).reshape(2, 128, 128))
    bias_bc = np.ascontiguousarray(
        np.broadcast_to(bias.astype(np.float16), (128, 128))
    )
    ident = np.eye(128, dtype=np.float16)
    iota16 = np.ascontiguousarray(
        np.broadcast_to(np.arange(128, dtype=np.float16), (128, 128))
    )

    row = np.asarray(edge_index[0]).astype(np.int64)
    col = np.asarray(edge_index[1]).astype(np.int64)

    # LPT-balance nodes into NBLK blocks of <=128 nodes (by degree) to
    # minimize the max edges-per-block.
    deg = np.bincount(row, minlength=N_NODES)
    order = np.argsort(-deg, kind="stable")
    blk_of = np.empty(N_NODES, np.int32)
    slot_of = np.empty(N_NODES, np.int32)
    heap = [(0, b) for b in range(NBLK)]
    heapq.heapify(heap)
    nslots = np.zeros(NBLK, np.int32)
    for n in order:
        load, b = heapq.heappop(heap)
        blk_of[n] = b
        slot_of[n] = nslots[b]
        nslots[b] += 1
        if nslots[b] < 128:
            heapq.heappush(heap, (load + int(deg[n]), b))

    b_of_edge = blk_of[row]
    eorder = np.argsort(b_of_edge, kind="stable")
    cs = col[eorder].astype(np.int32)
    rloc = slot_of[row[eorder]].astype(np.int32)
    counts = np.bincount(b_of_edge, minlength=NBLK)
    starts = np.concatenate([[0], np.cumsum(counts)])

    # Build one dedup'd stream per block pair (A = even slot, B = odd slot).
    # pairs[t of core c] = (global block id A, global block id B or -1)
    pair_blocks = []
    for c in range(NCORES):
        ids = list(range(c * NB, min((c + 1) * NB, NBLK)))
        ids += [-1] * (NB - len(ids))
        for t in range(NPAIR):
            pair_blocks.append((ids[2 * t], ids[2 * t + 1]))

    def blk_edges(g):
        if g < 0:
            return (np.empty(0, np.int32), np.empty(0, np.int32))
        e0, e1 = int(starts[g]), int(starts[g + 1])
        return cs[e0:e1], rloc[e0:e1]

    streams = []   # per pair: (cols, eA_pos, eA_rank, eA_rl, eB_...) arrays
    dims = []      # per pair: (n_only_a, n_a, n_stream)
    for gA, gB in pair_blocks:
        cA, rA = blk_edges(gA)
        cB, rB = blk_edges(gB)
        cp = np.concatenate([cA, cB])
        rp = np.concatenate([rA, rB])
        fp = np.concatenate(
            [np.zeros(cA.size, np.int8), np.ones(cB.size, np.int8)]
        )
        if cp.size == 0:
            streams.append((np.empty(0, np.int32),) + (np.empty(0, np.int64),) * 6)
            dims.append((0, 0, 0))
            continue
        o2 = np.argsort(cp, kind="stable")
        sc, sr, sf = cp[o2], rp[o2], fp[o2]
        new = np.r_[True, sc[1:] != sc[:-1]]
        gid = np.cumsum(new) - 1
        first = np.flatnonzero(new)
        ng = first.size
        dA = np.bincount(gid, weights=1 - sf.astype(np.int64), minlength=ng)
        dB = np.bincount(gid, weights=sf.astype(np.int64), minlength=ng)
        dA = dA.astype(np.int64)
        dB = dB.astype(np.int64)
        rank = np.arange(sc.size) - first[gid]       # A entries lead (stable)
        prank = np.where(sf == 0, rank, rank - dA[gid])
        # region 0: A-only, 1: shared, 2: B-only; sort by key desc within
        region = np.where(dB == 0, 0, np.where(dA == 0, 2, 1))
        key = np.where(region == 0, dA, np.where(region == 2, dB, np.maximum(dA, dB)))
        uorder = np.lexsort((-key, region))
        pos_of_group = np.empty(ng, np.int64)
        pos_of_group[uorder] = np.arange(ng)
        e_pos = pos_of_group[gid]
        isA = sf == 0
        streams.append((
            sc[first[uorder]].astype(np.int32),
            e_pos[isA], prank[isA], sr[isA].astype(np.int64),
            e_pos[~isA], prank[~isA], sr[~isA].astype(np.int64),
        ))
        dims.append((
            int((region[uorder] == 0).sum()),
            int((region[uorder] <= 1).sum()),
            ng,
        ))

    only_a = np.array([d[0] for d in dims])
    n_a = np.array([d[1] for d in dims])
    n_s = np.array([d[2] for d in dims])
    ra = int(np.max((n_a + 127) // 128))            # A-range chunks [0, ra)
    sa = int(np.min(only_a[n_s > 0] // 128))        # B-range start chunk
    pc = int(np.max((n_s + 127) // 128))            # chunks per pair stream
    pc = max(pc, ra, sa + 1)

    # Static pass schedules: global per-chunk max of dst-counts.
    pA = np.zeros(ra, np.int64)
    pB = np.zeros(pc - sa, np.int64)
    for (colsu, ap, ar, arl, bp, br, brl), _ in zip(streams, dims):
        if ap.size:
            np.maximum.at(pA, ap // 128, ar + 1)
        if bp.size:
            np.maximum.at(pB, bp // 128 - sa, br + 1)
    passes_a = np.maximum(pA, 1)
    passes_b = np.maximum(pB, 1)
    pcum_a = np.concatenate([[0], np.cumsum(passes_a)])
    pcum_b = np.concatenate([[0], np.cumsum(passes_b)])
    pa_tot, pb_tot = int(pcum_a[-1]), int(pcum_b[-1])
    pbs2 = pa_tot + pb_tot

    nidx = pc * 128
    idxc = nidx // 16
    col16 = np.zeros((NCORES, 128, NPAIR * idxc), np.int16)
    rloc16 = np.full((NCORES, 128, NPAIR * pbs2), -1.0, np.float32)
    for c in range(NCORES):
        for t in range(NPAIR):
            pi = c * NPAIR + t
            colsu, ap, ar, arl, bp, br, brl = streams[pi]
            lin_col = np.zeros(nidx, np.int32)
            lin_col[:colsu.size] = colsu
            lin_rl = np.full((pbs2, 128), -1.0, np.float32)
            if ap.size:
                lin_rl[pcum_a[ap // 128] + ar, ap % 128] = arl
            if bp.size:
                lin_rl[pa_tot + pcum_b[bp // 128 - sa] + br, bp % 128] = brl
            # the SWDGE tx/rx Q7 pair read the indices from different
            # 16-partition groups - replicate the 16-row wrap to all 128
            col16[c, :, t * idxc:(t + 1) * idxc] = np.tile(
                lin_col.reshape(idxc, 16).T.astype(np.int16), (8, 1)
            )
            rloc16[c, :, t * pbs2:(t + 1) * pbs2] = lin_rl.T
    # out_concat[blk*128 + slot] -> node (block ids are already slot order)
    pos = (blk_of * 128 + slot_of).astype(np.int64)
    meta = (pc, sa, ra, [int(v) for v in passes_a], [int(v) for v in passes_b])
    return x16, w_t, bias_bc, ident, iota16, col16, rloc16, meta, pos


def _build_program(meta):
    pc, sa, ra, passes_a, passes_b = meta
    pcum_a = [0]
    for v in passes_a:
        pcum_a.append(pcum_a[-1] + v)
    pcum_b = [0]
    for v in passes_b:
        pcum_b.append(pcum_b[-1] + v)
    pa_tot, pb_tot = pcum_a[-1], pcum_b[-1]
    pbs2 = pa_tot + pb_tot
    idxc = pc * 8
    nch = NPAIR * pc                 # global chunk count
    ngat = (nch + GRP - 1) // GRP    # flat gather instructions

    # range of block-slot q (pair t = q//2): list of (chunk j, passes, rlbase)
    def q_range(q):
        t = q // 2
        if q % 2 == 0:
            return [
                (t * pc + u, passes_a[u], t * pbs2 + pcum_a[u])
                for u in range(ra)
            ]
        return [
            (t * pc + sa + i, passes_b[i], t * pbs2 + pa_tot + pcum_b[i])
            for i in range(pc - sa)
        ]

    def smm_after(q):  # s_smm value after block-slot q's range completes
        t = q // 2
        return t * pbs2 + pa_tot if q % 2 == 0 else (t + 1) * pbs2

    def consume_tgt(j):  # s_smm value at which chunk j is fully consumed
        t, u = j // pc, j % pc
        if u < sa:
            return t * pbs2 + pcum_a[u + 1]
        return t * pbs2 + pa_tot + pcum_b[u - sa + 1]

    nc = bacc.Bacc("TRN2")

    x_d = nc.dram_tensor("x16", [N_NODES, FIN], FP16, kind="ExternalInput")
    w_d = nc.dram_tensor("w", [2, 128, 128], FP16, kind="ExternalInput")
    bb_d = nc.dram_tensor("bb", [128, 128], FP16, kind="ExternalInput")
    id_d = nc.dram_tensor("ident", [128, 128], FP16, kind="ExternalInput")
    io_d = nc.dram_tensor("iota", [128, 128], FP16, kind="ExternalInput")
    col_d = nc.dram_tensor("col", [128, NPAIR * idxc], I16, kind="ExternalInput")
    rl_d = nc.dram_tensor("rl", [128, NPAIR * pbs2], FP32, kind="ExternalInput")
    o_d = nc.dram_tensor("out", [NB * 128, 128], FP16, kind="ExternalOutput")

    from contextlib import ExitStack

    with ExitStack() as es:
        # aggT accumulators: [parity][feature-half], one bank each
        pa = [
            [es.enter_context(nc.psum_tensor(f"pa{k}{h}", [128, 512], FP32))
             for h in range(2)]
            for k in range(2)
        ]
        po = [es.enter_context(nc.psum_tensor(f"po{k}", [128, 512], FP32))
              for k in range(2)]
        w_sb = es.enter_context(nc.sbuf_tensor("w_sb", [128, 2, 128], FP16))
        bb_sb = es.enter_context(nc.sbuf_tensor("bb_sb", [128, 128], FP16))
        id_sb = es.enter_context(nc.sbuf_tensor("id_sb", [128, 128], FP16))
        iota_sb = es.enter_context(nc.sbuf_tensor("iota_sb", [128, 128], FP16))
        col_sb = es.enter_context(
            nc.sbuf_tensor("col_sb", [128, NPAIR * idxc], I16)
        )
        rl_sb = es.enter_context(
            nc.sbuf_tensor("rl_sb", [128, NPAIR * pbs2], FP32)
        )
        val_sb = es.enter_context(nc.sbuf_tensor("val_sb", [128, VC, FIN], FP16))
        s_sb = es.enter_context(nc.sbuf_tensor("s_sb", [128, S_BUFS, 128], FP16))
        at_sb = es.enter_context(nc.sbuf_tensor("at_sb", [128, 2, 2, 128], FP16))
        o_sb = es.enter_context(nc.sbuf_tensor("o_sb", [128, 2, 128], FP16))

        s_ld = [es.enter_context(nc.semaphore(f"s_ld{k}")) for k in range(9)]
        s_gat = [
            es.enter_context(nc.semaphore(f"s_gat{k}")) for k in range(NSEM)
        ]
        s_ow = [es.enter_context(nc.semaphore(f"s_ow{k}")) for k in range(2)]
        s_s = es.enter_context(nc.semaphore("s_s"))      # DVE one-hot count
        s_smm = es.enter_context(nc.semaphore("s_smm"))  # PE pass-mm count
        s_vcp = es.enter_context(nc.semaphore("s_vcp"))  # DVE aggT copies
        s_omm = es.enter_context(nc.semaphore("s_omm"))  # PE final-mm count
        s_ocp = es.enter_context(nc.semaphore("s_ocp"))  # ACT relu count
        block = es.enter_context(nc.Block())

        (LD_COL0, LD_COL1, LD_IO, LD_RL0, LD_RL1, LD_W, LD_W1, LD_BB,
         LD_ID) = range(9)

        @block.sync
        def _(sync):
            # Ramp-critical loads first: pair-0 idx slice gates the first
            # gather; iota + pair-0 rloc gate the first one-hot builds.
            sync.dma_start(
                col_sb[:, 0:idxc], col_d[:, 0:idxc]
            ).then_inc(s_ld[LD_COL0], 16)
            sync.dma_start(iota_sb[:, :], io_d[:, :]).then_inc(s_ld[LD_IO], 16)
            sync.dma_start(
                rl_sb[:, 0:pbs2], rl_d[:, 0:pbs2]
            ).then_inc(s_ld[LD_RL0], 16)
            sync.dma_start(
                col_sb[:, idxc:], col_d[:, idxc:]
            ).then_inc(s_ld[LD_COL1], 16)
            sync.dma_start(
                rl_sb[:, pbs2:], rl_d[:, pbs2:]
            ).then_inc(s_ld[LD_RL1], 16)
            sync.dma_start(w_sb[:, 0, :], w_d[0]).then_inc(s_ld[LD_W], 16)
            sync.dma_start(w_sb[:, 1, :], w_d[1]).then_inc(s_ld[LD_W1], 16)
            sync.dma_start(bb_sb[:, :], bb_d[:, :]).then_inc(s_ld[LD_BB], 16)
            sync.dma_start(id_sb[:, :], id_d[:, :]).then_inc(s_ld[LD_ID], 16)
            for b in range(NB):
                sync.wait_ge(s_ocp, b + 1)
                sync.dma_start(
                    o_d[b * 128:(b + 1) * 128, :], o_sb[:, b % 2, :]
                ).then_inc(s_ow[b % 2], 16)

        @block.gpsimd
        def _(gpsimd):
            gpsimd.wait_ge(s_ld[LD_COL0], 16)
            g_cross = next(g for g in range(ngat + 1) if GRP * g + GRP > pc)
            for g in range(ngat):
                if g == g_cross:
                    gpsimd.wait_ge(s_ld[LD_COL1], 16)
                j0, j1 = GRP * g, min(GRP * g + GRP, nch)
                if j1 - VC > 0:
                    # val ring slots [j0%VC, ...) held chunks [j0-VC, j1-VC)
                    gpsimd.wait_ge(s_smm, consume_tgt(j1 - VC - 1))
                r = j0 % VC
                gpsimd.dma_gather(
                    val_sb[:, r:r + (j1 - j0), :],
                    x_d[:, :],
                    col_sb[:, j0 * 8:j1 * 8],
                    (j1 - j0) * 128,
                    (j1 - j0) * 128,
                    FIN,
                ).then_inc(s_gat[g % NSEM], 16)

        def pe_final(tensor, q):
            if q == 0:
                for k in (LD_W, LD_W1, LD_BB, LD_ID):
                    tensor.wait_ge(s_ld[k], 16)
            tensor.wait_ge(s_vcp, 2 * (q + 1))
            if q >= 2:
                tensor.wait_ge(s_ocp, q - 1)
            tensor.matmul(
                po[q % 2][:, 0:128], id_sb[:, :], bb_sb[:, :],
                start=True, stop=False,
            )
            tensor.matmul(
                po[q % 2][:, 0:128], at_sb[:, q % 2, 0, :], w_sb[:, 0, :],
                start=False, stop=False,
            )
            tensor.matmul(
                po[q % 2][:, 0:128], at_sb[:, q % 2, 1, :], w_sb[:, 1, :],
                start=False, stop=True,
            ).then_inc(s_omm, 1)

        @block.tensor
        def _(tensor):
            kk = 0
            waited_g = 0
            for q in range(NB):
                rng = q_range(q)
                for i, (j, np_, rlb) in enumerate(rng):
                    g = j // GRP
                    while waited_g <= g:
                        tensor.wait_ge(
                            s_gat[waited_g % NSEM],
                            16 * (waited_g // NSEM + 1),
                        )
                        waited_g += 1
                    if i == 0 and q >= 2:
                        # pa[q%2] fully copied out (block-slot q-2)
                        tensor.wait_ge(s_vcp, 2 * (q - 1))
                    for p in range(np_):
                        tensor.wait_ge(s_s, kk + 1)
                        st = i == 0 and p == 0
                        sp = i == len(rng) - 1 and p == np_ - 1
                        tensor.matmul(
                            pa[q % 2][0][:, 0:128],
                            val_sb[:, j % VC, 0:128],
                            s_sb[:, kk % S_BUFS, :],
                            start=st,
                            stop=sp,
                        )
                        tensor.matmul(
                            pa[q % 2][1][:, 0:128],
                            val_sb[:, j % VC, 128:256],
                            s_sb[:, kk % S_BUFS, :],
                            start=st,
                            stop=sp,
                        ).then_inc(s_smm, 1)
                        kk += 1
                if q >= 1:
                    pe_final(tensor, q - 1)
            pe_final(tensor, NB - 1)

        def dve_copies(vector, q):
            vector.wait_ge(s_smm, smm_after(q))
            if q >= 2:
                # at_sb[q%2] consumed by pe_final(q-2)
                vector.wait_ge(s_omm, q - 1)
            vector.tensor_copy(
                at_sb[:, q % 2, 0, :], pa[q % 2][0][:, 0:128]
            ).then_inc(s_vcp, 1)
            vector.tensor_copy(
                at_sb[:, q % 2, 1, :], pa[q % 2][1][:, 0:128]
            ).then_inc(s_vcp, 1)

        @block.vector
        def _(vector):
            vector.wait_ge(s_ld[LD_IO], 16)
            vector.wait_ge(s_ld[LD_RL0], 16)
            kk = 0
            for q in range(NB):
                if q == 2:
                    vector.wait_ge(s_ld[LD_RL1], 16)
                for i, (j, np_, rlb) in enumerate(q_range(q)):
                    for p in range(np_):
                        if kk >= S_BUFS:
                            vector.wait_ge(s_smm, kk - S_BUFS + 1)
                        slot = rlb + p
                        vector.tensor_scalar(
                            s_sb[:, kk % S_BUFS, :],
                            iota_sb[:, :],
                            rl_sb[:, slot:slot + 1],
                            None,
                            mybir.AluOpType.is_equal,
                        ).then_inc(s_s, 1)
                        kk += 1
                if q >= 1:
                    dve_copies(vector, q - 1)
            dve_copies(vector, NB - 1)

        @block.scalar
        def _(scalar):
            for q in range(NB):
                scalar.wait_ge(s_omm, q + 1)
                if q >= 2:
                    scalar.wait_ge(s_ow[q % 2], 16 * (q // 2))
                scalar.activation(
                    o_sb[:, q % 2, :],
                    po[q % 2][:, 0:128],
                    mybir.ActivationFunctionType.Relu,
                ).then_inc(s_ocp, 1)

    nc.compile()
    return nc


def _run(x, edge_index, weight, bias, trace=False):
    x16, w_t, bias_bc, ident, iota16, col16, rloc16, meta, pos = _host_prep(
        x, edge_index, weight, bias
    )
    nc = _build_program(meta)
    in_maps = [
        {
            "x16": x16,
            "w": w_t,
            "bb": bias_bc,
            "ident": ident,
            "iota": iota16,
            "col": np.ascontiguousarray(col16[c]),
            "rl": np.ascontiguousarray(rloc16[c]),
        }
        for c in range(NCORES)
    ]
    res = run_bass_kernel_spmd(nc, in_maps, list(range(NCORES)), trace=trace)
    out = np.concatenate([res.results[c]["out"] for c in range(NCORES)], axis=0)
    return np.ascontiguousarray(out[pos].astype(np.float32)), res


def kernel(x, edge_index, weight, bias):
    out, _ = _run(x, edge_index, weight, bias, trace=False)
    return out


# revision 22
# speedup vs baseline: 3.1651x; 1.0217x over previous
"""GNN message-passing (graph convolution) kernel for 8 Trainium2 NeuronCores.

    out = relu(segment_sum(h[col], row) + bias),  h = x @ W

Strategy (v4, "aggregate-x-then-matmul" + paired-block dedup): by linearity,
segment_sum(x@W [col], row) = segment_sum(x[col], row) @ W, so the dense
projection is applied AFTER aggregation and the per-edge work is pure data
movement:

  * Host LPT-balances the 20000 nodes into 157 dst blocks of 128 (by degree);
    blocks are assigned contiguously to cores (20/core), so each core
    produces a disjoint output slice - no collectives.
  * Blocks are processed in PAIRS (A,B) sharing one gather stream laid out
    [A-only | A-and-B | B-only]: each distinct source column of the pair is
    gathered ONCE (block-level dedup plus pair-level sharing, ~17% fewer
    rows than raw edges).  A's one-hot matmul range covers the first part of
    the stream, B's the last; the shared middle is consumed by both.
    Sources with k>=2 destinations inside one block run k one-hot passes and
    are sorted to the front of their region (the static per-chunk pass count
    is the global max, so the SPMD instruction stream is identical on every
    core).
  * The gather stream is FLAT: SWDGE dma_gather instructions of 7 chunks
    (896 rows, the ring limit) are issued back to back into an 84-chunk val
    ring, crossing pair boundaries, minimizing the count of gather
    instructions (994ns fixed cost each on the GpSimd engine).  x rows
    (512B fp16) are gathered straight from DRAM.
  * DVE builds one-hot tiles S[e,n] = (iota == rowloc[pass]) in fp16; the PE
    accumulates aggT[f,n] += val[:,c,f]^T @ S over the block's range in PSUM
    fp32 - an exact transposed segment-sum (no PE transposes).
  * DVE copies aggT to SBUF fp16; PE computes out_b = aggT^T @ W + bias
    (bias via an identity-matmul against a broadcast bias tile); ACT applies
    ReLU (fp16 out); DMA out; host re-permutes and upcasts.

Numerics: fp16 operands with fp32 accumulation; one-hot matmuls are exact, so
the only error is fp16 rounding of x, W and the aggregate (~1e-3 relative).
"""

import sys

import numpy as np

sys.path.insert(0, "/opt/trn_rl_repo")

import concourse.bacc as bacc  # noqa: E402
import concourse.bass as bass  # noqa: E402  (engine types)
import concourse.mybir as mybir  # noqa: E402
from concourse.bass_utils import run_bass_kernel_spmd  # noqa: E402

N_NODES = 20000
FIN = 256
FOUT = 128
N_EDGES = 640000

NBLK = 157               # dst blocks of 128 nodes (157*128 = 20096 slots)
NCORES = 8
NB = 20                  # block slots per core (core 7: 17 real + 3 dummy)
NPAIR = NB // 2

S_BUFS = 8               # one-hot tile ring
GRP = 7                  # chunks per gather (896 rows <= 1024-desc SWDGE ring)
VC = 84                  # val ring chunks (multiple of GRP)
NSEM = VC // GRP         # rotating gather-completion semaphores

FP16 = mybir.dt.float16
FP32 = mybir.dt.float32
I16 = mybir.dt.int16


def _host_prep(x, edge_index, weight, bias):
    """Cast operands, balance nodes into blocks, build paired gather streams."""
    import heapq

    x16 = np.ascontiguousarray(np.asarray(x, np.float32).astype(np.float16))
    weight = np.asarray(weight, np.float32)
    bias = np.asarray(bias, np.float32)

    w_t = np.ascontiguousarray(weight.astype(np.float16).reshape(2, 128, 128))
    bias_bc = np.ascontiguousarray(
        np.broadcast_to(bias.astype(np.float16), (128, 128))
    )
    ident = np.eye(128, dtype=np.float16)
    iota16 = np.ascontiguousarray(
        np.broadcast_to(np.arange(128, dtype=np.float16), (128, 128))
    )

    row = np.asarray(edge_index[0]).astype(np.int64)
    col = np.asarray(edge_index[1]).astype(np.int64)

    # LPT-balance nodes into NBLK blocks of <=128 nodes (by degree) to
    # minimize the max edges-per-block.
    deg = np.bincount(row, minlength=N_NODES)
    order = np.argsort(-deg, kind="stable")
    blk_of = np.empty(N_NODES, np.int32)
    slot_of = np.empty(N_NODES, np.int32)
    heap = [(0, b) for b in range(NBLK)]
    heapq.heapify(heap)
    nslots = np.zeros(NBLK, np.int32)
    for n in order:
        load, b = heapq.heappop(heap)
        blk_of[n] = b
        slot_of[n] = nslots[b]
        nslots[b] += 1
        if nslots[b] < 128:
            heapq.heappush(heap, (load + int(deg[n]), b))

    b_of_edge = blk_of[row]
    eorder = np.argsort(b_of_edge, kind="stable")
    cs = col[eorder].astype(np.int32)
    rloc = slot_of[row[eorder]].astype(np.int32)
    counts = np.bincount(b_of_edge, minlength=NBLK)
    starts = np.concatenate([[0], np.cumsum(counts)])

    # Build one dedup'd CHAIN stream per core: segments seg_0..seg_19, one
    # per block slot.  A column of block q already emitted in seg q-1 (and
    # thus covered by range q) is not re-emitted; emitted columns also in
    # block q+1 form the segment's "shared" tail, consumed by range q+1 too.
    def blk_edges(g):
        if g < 0 or g >= NBLK:
            return (np.empty(0, np.int32), np.empty(0, np.int32))
        e0, e1 = int(starts[g]), int(starts[g + 1])
        return cs[e0:e1], rloc[e0:e1]

    # Pre-group every block's edges by column.
    ublk = []
    for g in range(NBLK):
        c_b, r_b = blk_edges(g)
        o2 = np.argsort(c_b, kind="stable")
        sc, sr = c_b[o2], r_b[o2]
        new = np.r_[True, sc[1:] != sc[:-1]] if sc.size else np.empty(0, bool)
        first = np.flatnonzero(new)
        gid = np.cumsum(new) - 1 if sc.size else np.empty(0, np.int64)
        dcnt = np.diff(np.r_[first, sc.size]) if sc.size else np.empty(0, np.int64)
        rank = (np.arange(sc.size) - first[gid]) if sc.size else np.empty(0, np.int64)
        ublk.append((sc[first] if sc.size else np.empty(0, np.int32),
                     dcnt, gid, rank, sr))

    E = np.empty(0, np.int64)
    seg_n = np.zeros((NCORES, NB), np.int64)    # rows per segment
    seg_only = np.zeros((NCORES, NB), np.int64)  # rows before shared tail
    seg_cols = {}                                # (c,q) -> col values in order
    entries = {}   # (c,q) -> (seg_of_row, u=row//128, part, rank, rl)
    for c in range(NCORES):
        prev_cols = np.empty(0, np.int32)        # emitted shared cols of q-1
        prev_pos = np.empty(0, np.int64)         # their row pos in seg q-1
        for q in range(NB):
            g = c * NB + q
            if g >= NBLK:
                seg_cols[(c, q)] = np.empty(0, np.int32)
                entries[(c, q)] = (E, E, E, E, E)
                prev_cols, prev_pos = np.empty(0, np.int32), E
                continue
            ucols, dcnt, gid, rank, sr = ublk[g]
            ng = ucols.size
            covered = np.isin(ucols, prev_cols)
            emit_idx = np.flatnonzero(~covered)
            emit_cols = ucols[emit_idx]
            gn = g + 1 if (q + 1 < NB and g + 1 < NBLK) else -1
            if gn >= 0:
                nxt_ucols, nxt_dcnt = ublk[gn][0], ublk[gn][1]
                ip = np.searchsorted(nxt_ucols, emit_cols)
                ip = np.minimum(ip, max(nxt_ucols.size - 1, 0))
                in_next = (nxt_ucols.size > 0) & (nxt_ucols[ip] == emit_cols)
                d_next = np.where(in_next, nxt_dcnt[ip], 0)
            else:
                in_next = np.zeros(emit_cols.size, bool)
                d_next = np.zeros(emit_cols.size, np.int64)
            d_cur = dcnt[emit_idx]
            # share only single-dst columns: keeps the static pass schedule
            # flat (1) over shared tails instead of inflating every range
            in_next = in_next & (d_cur == 1) & (d_next == 1)
            key = np.where(in_next, np.maximum(d_cur, d_next), d_cur)
            uo = np.lexsort((-key, in_next.astype(np.int8)))
            pos_of_emit = np.empty(emit_cols.size, np.int64)
            pos_of_emit[uo] = np.arange(emit_cols.size)
            seg_cols[(c, q)] = emit_cols[uo]
            seg_n[c, q] = emit_cols.size
            seg_only[c, q] = int((~in_next).sum())
            # per-group row position: covered -> prev seg, else this seg
            grow = np.empty(ng, np.int64)
            gseg = np.empty(ng, np.int64)
            if prev_cols.size:
                pi = np.searchsorted(prev_cols, ucols[covered])
                grow[covered] = prev_pos[pi]
                gseg[covered] = q - 1
            grow[~covered] = pos_of_emit
            gseg[~covered] = q
            entries[(c, q)] = (
                gseg[gid], grow[gid] // 128, grow[gid] % 128,
                rank, sr.astype(np.int64),
            )
            shared = np.flatnonzero(in_next[uo])
            prev_cols = emit_cols[uo][shared]
            po = np.argsort(prev_cols, kind="stable")
            prev_cols = prev_cols[po]
            prev_pos = shared[po]

    segc = np.maximum(seg_n.max(axis=0) + 127, 128) // 128  # chunks per seg
    posq = np.concatenate([[0], np.cumsum(segc)])
    tc = int(posq[-1])                                       # chunks per core
    rs, re = [0] * NB, [0] * NB
    for q in range(NB):
        re[q] = int(posq[q] + segc[q])
        if q == 0:
            rs[q] = 0
        else:
            live = seg_n[:, q - 1] > 0
            off = int((seg_only[live, q - 1] // 128).min()) if live.any() else \
                int(segc[q - 1])
            rs[q] = int(posq[q - 1]) + off

    # Static pass schedules: per-range per-chunk max of dst-counts.
    passes = []
    for q in range(NB):
        pq = np.zeros(re[q] - rs[q], np.int64)
        for c in range(NCORES):
            gseg, u, part, rank, rl = entries[(c, q)]
            if len(gseg) == 0:
                continue
            i = posq[gseg] + u - rs[q]
            np.maximum.at(pq, i, rank + 1)
        passes.append(np.maximum(pq, 1))
    pcums = [np.concatenate([[0], np.cumsum(p)]) for p in passes]
    sbase = np.concatenate([[0], np.cumsum([int(p[-1]) for p in pcums])])
    tslots = int(sbase[-1])

    nidx = tc * 128
    idxc = nidx // 16
    col16 = np.zeros((NCORES, 128, idxc), np.int16)
    rloc16 = np.full((NCORES, 128, tslots), -1.0, np.float32)
    for c in range(NCORES):
        lin_col = np.zeros(nidx, np.int32)
        lin_rl = np.full((tslots, 128), -1.0, np.float32)
        for q in range(NB):
            u0 = seg_cols[(c, q)]
            lin_col[posq[q] * 128:posq[q] * 128 + u0.size] = u0
            gseg, u, part, rank, rl = entries[(c, q)]
            if len(gseg) == 0:
                continue
            i = posq[gseg] + u - rs[q]
            lin_rl[sbase[q] + pcums[q][i] + rank, part] = rl
        # the SWDGE Q7 cores read the indices from different 16-partition
        # groups - replicate the 16-row wrap to all 128
        col16[c] = np.tile(lin_col.reshape(idxc, 16).T.astype(np.int16), (8, 1))
        rloc16[c] = lin_rl.T
    # out_concat[blk*128 + slot] -> node (block ids are already slot order)
    pos = (blk_of * 128 + slot_of).astype(np.int64)
    meta = (tc, rs, re, [list(map(int, p)) for p in passes])
    return x16, w_t, bias_bc, ident, iota16, col16, rloc16, meta, pos


def _build_program(meta):
    tc, rs, re, passes = meta
    pcums = []
    for p in passes:
        c = [0]
        for v in p:
            c.append(c[-1] + v)
        pcums.append(c)
    sbase = [0]
    for c in pcums:
        sbase.append(sbase[-1] + c[-1])
    tslots = sbase[-1]
    idxc = tc * 8
    nch = tc                         # global chunk count
    ngat = (nch + GRP - 1) // GRP    # flat gather instructions

    # range of block-slot q: list of (chunk j, passes, rl slot base)
    def q_range(q):
        return [
            (rs[q] + i, passes[q][i], sbase[q] + pcums[q][i])
            for i in range(re[q] - rs[q])
        ]

    def smm_after(q):  # s_smm value after block-slot q's range completes
        return sbase[q + 1]

    # s_smm value at which chunk j is fully consumed (last covering range)
    tgt = [0] * nch
    for q in range(NB):
        for i in range(re[q] - rs[q]):
            tgt[rs[q] + i] = sbase[q] + pcums[q][i + 1]

    def consume_tgt(j):
        return tgt[j]

    # ramp split points: chunks/slots of the first two ranges
    ch0 = re[1]
    sl0 = sbase[2]

    nc = bacc.Bacc("TRN2")

    x_d = nc.dram_tensor("x16", [N_NODES, FIN], FP16, kind="ExternalInput")
    w_d = nc.dram_tensor("w", [2, 128, 128], FP16, kind="ExternalInput")
    bb_d = nc.dram_tensor("bb", [128, 128], FP16, kind="ExternalInput")
    id_d = nc.dram_tensor("ident", [128, 128], FP16, kind="ExternalInput")
    io_d = nc.dram_tensor("iota", [128, 128], FP16, kind="ExternalInput")
    col_d = nc.dram_tensor("col", [128, idxc], I16, kind="ExternalInput")
    rl_d = nc.dram_tensor("rl", [128, tslots], FP32, kind="ExternalInput")
    o_d = nc.dram_tensor("out", [NB * 128, 128], FP16, kind="ExternalOutput")

    from contextlib import ExitStack

    with ExitStack() as es:
        # aggT accumulators: [parity][feature-half], one bank each
        pa = [
            [es.enter_context(nc.psum_tensor(f"pa{k}{h}", [128, 512], FP32))
             for h in range(2)]
            for k in range(2)
        ]
        po = [es.enter_context(nc.psum_tensor(f"po{k}", [128, 512], FP32))
              for k in range(2)]
        w_sb = es.enter_context(nc.sbuf_tensor("w_sb", [128, 2, 128], FP16))
        bb_sb = es.enter_context(nc.sbuf_tensor("bb_sb", [128, 128], FP16))
        id_sb = es.enter_context(nc.sbuf_tensor("id_sb", [128, 128], FP16))
        iota_sb = es.enter_context(nc.sbuf_tensor("iota_sb", [128, 128], FP16))
        col_sb = es.enter_context(
            nc.sbuf_tensor("col_sb", [128, idxc], I16)
        )
        rl_sb = es.enter_context(
            nc.sbuf_tensor("rl_sb", [128, tslots], FP32)
        )
        val_sb = es.enter_context(nc.sbuf_tensor("val_sb", [128, VC, FIN], FP16))
        s_sb = es.enter_context(nc.sbuf_tensor("s_sb", [128, S_BUFS, 128], FP16))
        at_sb = es.enter_context(nc.sbuf_tensor("at_sb", [128, 2, 2, 128], FP16))
        o_sb = es.enter_context(nc.sbuf_tensor("o_sb", [128, 2, 128], FP16))

        s_ld = [es.enter_context(nc.semaphore(f"s_ld{k}")) for k in range(9)]
        s_gat = [
            es.enter_context(nc.semaphore(f"s_gat{k}")) for k in range(NSEM)
        ]
        s_ow = [es.enter_context(nc.semaphore(f"s_ow{k}")) for k in range(2)]
        s_s = es.enter_context(nc.semaphore("s_s"))      # DVE one-hot count
        s_smm = es.enter_context(nc.semaphore("s_smm"))  # PE pass-mm count
        s_vcp = es.enter_context(nc.semaphore("s_vcp"))  # DVE aggT copies
        s_omm = es.enter_context(nc.semaphore("s_omm"))  # PE final-mm count
        s_ocp = es.enter_context(nc.semaphore("s_ocp"))  # ACT relu count
        block = es.enter_context(nc.Block())

        (LD_COL0, LD_COL1, LD_IO, LD_RL0, LD_RL1, LD_W, LD_W1, LD_BB,
         LD_ID) = range(9)

        @block.sync
        def _(sync):
            # Ramp-critical loads first: pair-0 idx slice gates the first
            # gather; iota + pair-0 rloc gate the first one-hot builds.
            sync.dma_start(
                col_sb[:, 0:ch0 * 8], col_d[:, 0:ch0 * 8]
            ).then_inc(s_ld[LD_COL0], 16)
            sync.dma_start(iota_sb[:, :], io_d[:, :]).then_inc(s_ld[LD_IO], 16)
            sync.dma_start(
                rl_sb[:, 0:sl0], rl_d[:, 0:sl0]
            ).then_inc(s_ld[LD_RL0], 16)
            sync.dma_start(
                col_sb[:, ch0 * 8:], col_d[:, ch0 * 8:]
            ).then_inc(s_ld[LD_COL1], 16)
            sync.dma_start(
                rl_sb[:, sl0:], rl_d[:, sl0:]
            ).then_inc(s_ld[LD_RL1], 16)
            sync.dma_start(w_sb[:, 0, :], w_d[0]).then_inc(s_ld[LD_W], 16)
            sync.dma_start(w_sb[:, 1, :], w_d[1]).then_inc(s_ld[LD_W1], 16)
            sync.dma_start(bb_sb[:, :], bb_d[:, :]).then_inc(s_ld[LD_BB], 16)
            sync.dma_start(id_sb[:, :], id_d[:, :]).then_inc(s_ld[LD_ID], 16)
            for b in range(NB):
                sync.wait_ge(s_ocp, b + 1)
                sync.dma_start(
                    o_d[b * 128:(b + 1) * 128, :], o_sb[:, b % 2, :]
                ).then_inc(s_ow[b % 2], 16)

        @block.gpsimd
        def _(gpsimd):
            gpsimd.wait_ge(s_ld[LD_COL0], 16)
            g_cross = next(g for g in range(ngat + 1) if GRP * g + GRP > ch0)
            for g in range(ngat):
                if g == g_cross:
                    gpsimd.wait_ge(s_ld[LD_COL1], 16)
                j0, j1 = GRP * g, min(GRP * g + GRP, nch)
                if j1 - VC > 0:
                    # val ring slots [j0%VC, ...) held chunks [j0-VC, j1-VC)
                    gpsimd.wait_ge(s_smm, consume_tgt(j1 - VC - 1))
                r = j0 % VC
                gpsimd.dma_gather(
                    val_sb[:, r:r + (j1 - j0), :],
                    x_d[:, :],
                    col_sb[:, j0 * 8:j1 * 8],
                    (j1 - j0) * 128,
                    (j1 - j0) * 128,
                    FIN,
                ).then_inc(s_gat[g % NSEM], 16)

        def pe_final(tensor, q):
            if q == 0:
                for k in (LD_W, LD_W1, LD_BB, LD_ID):
                    tensor.wait_ge(s_ld[k], 16)
            tensor.wait_ge(s_vcp, 2 * (q + 1))
            if q >= 2:
                tensor.wait_ge(s_ocp, q - 1)
            tensor.matmul(
                po[q % 2][:, 0:128], id_sb[:, :], bb_sb[:, :],
                start=True, stop=False,
            )
            tensor.matmul(
                po[q % 2][:, 0:128], at_sb[:, q % 2, 0, :], w_sb[:, 0, :],
                start=False, stop=False,
            )
            tensor.matmul(
                po[q % 2][:, 0:128], at_sb[:, q % 2, 1, :], w_sb[:, 1, :],
                start=False, stop=True,
            ).then_inc(s_omm, 1)

        @block.tensor
        def _(tensor):
            kk = 0
            waited_g = 0
            for q in range(NB):
                rng = q_range(q)
                for i, (j, np_, rlb) in enumerate(rng):
                    g = j // GRP
                    while waited_g <= g:
                        tensor.wait_ge(
                            s_gat[waited_g % NSEM],
                            16 * (waited_g // NSEM + 1),
                        )
                        waited_g += 1
                    if i == 0 and q >= 2:
                        # pa[q%2] fully copied out (block-slot q-2)
                        tensor.wait_ge(s_vcp, 2 * (q - 1))
                    for p in range(np_):
                        tensor.wait_ge(s_s, kk + 1)
                        st = i == 0 and p == 0
                        sp = i == len(rng) - 1 and p == np_ - 1
                        tensor.matmul(
                            pa[q % 2][0][:, 0:128],
                            val_sb[:, j % VC, 0:128],
                            s_sb[:, kk % S_BUFS, :],
                            start=st,
                            stop=sp,
                        )
                        tensor.matmul(
                            pa[q % 2][1][:, 0:128],
                            val_sb[:, j % VC, 128:256],
                            s_sb[:, kk % S_BUFS, :],
                            start=st,
                            stop=sp,
                        ).then_inc(s_smm, 1)
                        kk += 1
                if q >= 1:
                    pe_final(tensor, q - 1)
            pe_final(tensor, NB - 1)

        def dve_copies(vector, q):
            vector.wait_ge(s_smm, smm_after(q))
            if q >= 2:
                # at_sb[q%2] consumed by pe_final(q-2)
                vector.wait_ge(s_omm, q - 1)
            vector.tensor_copy(
                at_sb[:, q % 2, 0, :], pa[q % 2][0][:, 0:128]
            ).then_inc(s_vcp, 1)
            vector.tensor_copy(
                at_sb[:, q % 2, 1, :], pa[q % 2][1][:, 0:128]
            ).then_inc(s_vcp, 1)

        @block.vector
        def _(vector):
            vector.wait_ge(s_ld[LD_IO], 16)
            vector.wait_ge(s_ld[LD_RL0], 16)
            kk = 0
            for q in range(NB):
                if q == 2:
                    vector.wait_ge(s_ld[LD_RL1], 16)
                for i, (j, np_, rlb) in enumerate(q_range(q)):
                    for p in range(np_):
                        if kk >= S_BUFS:
                            vector.wait_ge(s_smm, kk - S_BUFS + 1)
                        slot = rlb + p
                        vector.tensor_scalar(
                            s_sb[:, kk % S_BUFS, :],
                            iota_sb[:, :],
                            rl_sb[:, slot:slot + 1],
                            None,
                            mybir.AluOpType.is_equal,
                        ).then_inc(s_s, 1)
                        kk += 1
                if q >= 1:
                    dve_copies(vector, q - 1)
            dve_copies(vector, NB - 1)

        @block.scalar
        def _(scalar):
            for q in range(NB):
                scalar.wait_ge(s_omm, q + 1)
                if q >= 2:
                    scalar.wait_ge(s_ow[q % 2], 16 * (q // 2))
                scalar.activation(
                    o_sb[:, q % 2, :],
                    po[q % 2][:, 0:128],
                    mybir.ActivationFunctionType.Relu,
                ).then_inc(s_ocp, 1)

    nc.compile()
    return nc


def _run(x, edge_index, weight, bias, trace=False):
    x16, w_t, bias_bc, ident, iota16, col16, rloc16, meta, pos = _host_prep(
        x, edge_index, weight, bias
    )
    nc = _build_program(meta)
    in_maps = [
        {
            "x16": x16,
            "w": w_t,
            "bb": bias_bc,
            "ident": ident,
            "iota": iota16,
            "col": np.ascontiguousarray(col16[c]),
            "rl": np.ascontiguousarray(rloc16[c]),
        }
        for c in range(NCORES)
    ]
    res = run_bass_kernel_spmd(nc, in_maps, list(range(NCORES)), trace=trace)
    out = np.concatenate([res.results[c]["out"] for c in range(NCORES)], axis=0)
    return np.ascontiguousarray(out[pos].astype(np.float32)), res


def kernel(x, edge_index, weight, bias):
    out, _ = _run(x, edge_index, weight, bias, trace=False)
    return out
